# revision 1
# baseline (speedup 1.0000x reference)
"""RGCN (2-layer, mean-aggregation) Bass kernel for one TRN2 chip (8 NeuronCores).

Strategy (edge/dst-sharded, aggregate-then-transform):
  - Nodes are block-partitioned across the 8 cores (12500/core). Edges live on
    their *dst*-owner core, so all aggregation is core-local (no reduce).
  - x is replicated (bf16) in every core's HBM. Per edge: dma_gather x[src]
    (256B rows), scale by inv_deg (precomputed on host), dma_scatter_add into
    per-relation accumulators A_r[dst_local] (bf16, zero-initialized as
    ExternalOutputs).
  - Transform phase: out[dst] = relu(sum_r A_r @ W_r + x_local @ root + b),
    computed as 128x128 PSUM tiles; A_r^T tiles come in via HW DMA-transpose
    (bf16), weights are bf16, accumulation is f32 in PSUM.
  - Between layers, a single small AllGather (bf16, 3.2MB/rank) replicates the
    new features; layer-1's replication is free (host pre-stages x_rep).
  - int16 gather/scatter indices are kept in range by grouping edges by
    (src block, relation-pair): gather tables are the 12544-row node blocks of
    the replicated x; scatter targets are [2*12544, 128] relation-pair tables.
"""

import numpy as np
import ml_dtypes

import concourse.tile as tile
from concourse import bass, bacc, mybir
from concourse.bass_utils import run_bass_kernel_spmd

BF16 = mybir.dt.bfloat16
F32 = mybir.dt.float32
I16 = mybir.dt.int16
bf16 = ml_dtypes.bfloat16
import os
# max indices per gather/scatter call: single_packet=True caps at 1024
# (multi-call ring pressure), single_packet=False caps at 1920 (<2048 field)
SINGLE_PACKET = os.environ.get("K_SP", "0") == "1"
MAXC = 1024 if SINGLE_PACKET else 1920

# ----------------------------------------------------------------------------
# Problem constants (full-size; mini configs for sim tests override these)
# ----------------------------------------------------------------------------
FULL = dict(N=100000, E=1000000, D=128, R=8, C=8)


def derive(cfg):
    N, D, R, C = cfg["N"], cfg["D"], cfg["R"], cfg["C"]
    NL = N // C                      # owned nodes per core
    NT = (NL + 128) // 128           # node tiles per core (>= 1 dustbin row)
    NLP = NT * 128                   # padded rows per block
    RP = R // 2                      # relation-pair tables
    # transform chunk size (tiles per DMA-transpose batch)
    TCH = 14 if NT % 14 == 0 else (7 if NT % 7 == 0 else (3 if NT % 3 == 0 else 1))
    return NL, NT, NLP, RP, TCH


# ----------------------------------------------------------------------------
# Host-side preprocessing
# ----------------------------------------------------------------------------
def host_prep(x, edge_index, edge_type, cfg):
    """Sort/pad edges, build wrapped int16 index tiles and scale tiles."""
    N, E, D, R, C = cfg["N"], cfg["E"], cfg["D"], cfg["R"], cfg["C"]
    NL, NT, NLP, RP, TCH = derive(cfg)

    src = np.asarray(edge_index[0], dtype=np.int64)
    dst = np.asarray(edge_index[1], dtype=np.int64)
    et = np.asarray(edge_type, dtype=np.int64)

    # mean-normalization per (relation, dst), computed on host (graph-only)
    deg = np.zeros((R, N), np.float32)
    np.add.at(deg, (et, dst), 1.0)
    inv = np.where(deg > 0, 1.0 / np.maximum(deg, 1.0), 0.0).astype(np.float32)
    scale_e = inv[et, dst]

    core = dst // NL
    blk = src // NL
    rp = et // 2
    unit = blk * RP + rp                      # unit id within a core
    NU = C * RP                               # units per core
    # scatter index within the relation-pair table
    sidx_val = (et % 2) * NLP + (dst % NL)
    gidx_val = src % NL
    DUSTBIN = NL                              # pad rows add 0.0 to this row

    # The HW scatter-add loses updates when the same destination row appears
    # more than once in one call (concurrent RMW descriptors). Within each
    # (core, unit) group we therefore rank duplicate sidx occurrences and
    # emit one scatter sub-call per rank: indices are unique within a call,
    # and calls targeting the same table are serialized.
    order = np.lexsort((sidx_val, unit, core))
    key_core, key_unit, key_sidx = core[order], unit[order], sidx_val[order]
    new_run = np.ones(len(order), bool)
    new_run[1:] = (
        (key_core[1:] != key_core[:-1])
        | (key_unit[1:] != key_unit[:-1])
        | (key_sidx[1:] != key_sidx[:-1])
    )
    run_id = np.cumsum(new_run) - 1
    run_starts = np.flatnonzero(new_run)
    rank = np.arange(len(order)) - run_starts[run_id]

    # final order: (core, unit, rank, sidx)
    order2 = order[np.lexsort((key_sidx, rank, key_unit, key_core))]
    rank2 = rank[np.lexsort((key_sidx, rank, key_unit, key_core))]
    src_o, core_o, unit_o = gidx_val[order2], core[order2], unit[order2]
    sidx_o, scale_o = sidx_val[order2], scale_e[order2]

    NRANK = int(rank.max()) + 1
    # per-(core, unit, rank) counts; span size = max over cores, padded to 128
    counts = np.zeros((C, NU, NRANK), np.int64)
    np.add.at(counts, (core_o, unit_o, rank2), 1)
    spans = (np.ceil(counts.max(axis=0) / 128).astype(np.int64) * 128)  # [NU, NRANK]
    unit_sizes = spans.sum(axis=1)                                     # [NU]
    offs = np.zeros(NU + 1, np.int64)
    np.cumsum(unit_sizes, out=offs[1:])
    GT = int(offs[-1])                        # total padded edges per core

    gidx = np.zeros((C, GT), np.int16)
    sidx = np.full((C, GT), DUSTBIN, np.int16)
    scale = np.zeros((C, GT), np.float32)

    starts = np.zeros((C, NU, NRANK), np.int64)
    flat = counts.reshape(-1)
    np.cumsum(flat[:-1], out=starts.reshape(-1)[1:])
    for c in range(C):
        for u in range(NU):
            pos = offs[u]
            for j in range(NRANK):
                s, n = starts[c, u, j], counts[c, u, j]
                gidx[c, pos : pos + n] = src_o[s : s + n]
                sidx[c, pos : pos + n] = sidx_o[s : s + n]
                scale[c, pos : pos + n] = scale_o[s : s + n]
                pos += spans[u, j]

    # wrapped layouts (per 16 / per 128 within each unit's region):
    #  idx tiles: [128, G/16] int16, idx i at [i%16, i//16], replicated x8 down
    #  scale tiles: [128, G/128] bf16, edge i at [i%128, i//128]
    gidx_w = np.zeros((C, 128, GT // 16), np.int16)
    sidx_w = np.zeros((C, 128, GT // 16), np.int16)
    scale_w = np.zeros((C, 128, GT // 128), bf16)
    for u in range(NU):
        o, g = offs[u], unit_sizes[u]
        wi = gidx[:, o : o + g].reshape(C, g // 16, 16).transpose(0, 2, 1)
        gidx_w[:, :, o // 16 : (o + g) // 16] = np.tile(wi, (1, 8, 1))
        wi = sidx[:, o : o + g].reshape(C, g // 16, 16).transpose(0, 2, 1)
        sidx_w[:, :, o // 16 : (o + g) // 16] = np.tile(wi, (1, 8, 1))
        ws = scale[:, o : o + g].reshape(C, g // 128, 128).transpose(0, 2, 1)
        scale_w[:, :, o // 128 : (o + g) // 128] = ws.astype(bf16)

    # replicated, block-padded x (bf16): [C*NLP, D]
    x = np.asarray(x, np.float32)
    x_rep = np.zeros((C * NLP, D), bf16)
    for c in range(C):
        x_rep[c * NLP : c * NLP + NL] = x[c * NL : (c + 1) * NL].astype(bf16)

    return dict(
        spans=tuple(tuple(int(v) for v in row if v) for row in spans),
        NU=NU,
        gidx=np.ascontiguousarray(gidx_w),
        sidx=np.ascontiguousarray(sidx_w),
        scale=np.ascontiguousarray(scale_w),
        x_rep=x_rep,
    )


# ----------------------------------------------------------------------------
# Device program
# ----------------------------------------------------------------------------
def build_program(cfg, spans, NU):
    N, E, D, R, C = cfg["N"], cfg["E"], cfg["D"], cfg["R"], cfg["C"]
    NL, NT, NLP, RP, TCH = derive(cfg)
    unit_sizes = [sum(row) for row in spans]
    offs = [0]
    for g in unit_sizes:
        offs.append(offs[-1] + g)
    GT = offs[-1]                        # total padded edges per core
    NCH = NT // TCH                      # transform chunks

    nc = bacc.Bacc(
        "TRN2", target_bir_lowering=False, debug=False,
        enable_asserts=False, num_devices=C,
    )

    # ---- I/O ----
    x_rep = nc.dram_tensor("x_rep", [C * NLP, D], BF16, kind="ExternalInput")
    x_loc = nc.dram_tensor("x_loc", [NLP, D], BF16, kind="ExternalInput")
    w_all = nc.dram_tensor("w_all", [2, R + 1, D, D], BF16, kind="ExternalInput")
    b_all = nc.dram_tensor("b_all", [2, 1, D], BF16, kind="ExternalInput")
    gidx_d = nc.dram_tensor("gidx", [128, GT // 16], I16, kind="ExternalInput")
    sidx_d = nc.dram_tensor("sidx", [128, GT // 16], I16, kind="ExternalInput")
    scale_d = nc.dram_tensor("scale", [128, GT // 128], BF16, kind="ExternalInput")

    # zero-initialized accumulators (ExternalOutputs are pre-zeroed)
    A = [
        [nc.dram_tensor(f"A_{l}_{p}", [2 * NLP, D], BF16, kind="ExternalOutput")
         for p in range(RP)]
        for l in range(2)
    ]
    out_d = nc.dram_tensor("out", [NL, D], F32, kind="ExternalOutput")

    # internal buffers for the inter-layer AllGather
    h1b = nc.dram_tensor("h1b", [NLP, D], BF16, kind="Internal")
    h1rep = nc.dram_tensor(
        "h1rep", [C * NLP, D], BF16, kind="Internal", addr_space="Shared"
    )

    with tile.TileContext(nc) as tc:
        with (
            tc.tile_pool(name="resident", bufs=1) as res_pool,
            tc.tile_pool(name="gather", bufs=3) as gpool,
            tc.tile_pool(name="station", bufs=2) as spool,
            tc.tile_pool(name="wpool", bufs=1) as wpool,
            tc.tile_pool(name="hout", bufs=4) as hpool,
            tc.tile_pool(name="psum", bufs=4, space="PSUM") as psum_pool,
        ):
            # resident index/scale tiles (shared by both layers)
            gidx_sb = res_pool.tile([128, GT // 16], I16)
            sidx_sb = res_pool.tile([128, GT // 16], I16)
            scale_sb = res_pool.tile([128, GT // 128], BF16)
            nc.sync.dma_start(out=gidx_sb[:], in_=gidx_d.ap()[:, :])
            nc.sync.dma_start(out=sidx_sb[:], in_=sidx_d.ap()[:, :])
            nc.sync.dma_start(out=scale_sb[:], in_=scale_d.ap()[:, :])

            ones_sb = res_pool.tile([1, D], BF16)
            nc.vector.memset(ones_sb[:], 1.0)

            for lay in range(2):
                src_tab = x_rep if lay == 0 else h1rep
                loc_tab = x_loc if lay == 0 else h1b

                # ---- edge phase: gather -> scale -> scatter-add ----
                es = nc.enter_named_scope(f"edge_{lay}", False)
                for u in range(NU):
                    blk, rpi = u // RP, u % RP
                    G = unit_sizes[u]
                    o = offs[u]
                    g = gpool.tile([128, G // 128, D], BF16, tag="g")
                    # HW caps one gather/scatter call at <2048 indices
                    for co in range(0, G, MAXC):
                        n = min(MAXC, G - co)
                        nc.gpsimd.dma_gather(
                            out_ap=g[:, co // 128 : (co + n) // 128, :],
                            in_ap=src_tab.ap()[blk * NLP : blk * NLP + NLP, :],
                            idxs_ap=gidx_sb[:, (o + co) // 16 : (o + co + n) // 16],
                            num_idxs=n,
                            num_idxs_reg=n,
                            elem_size=D,
                            single_packet=SINGLE_PACKET,
                        )
                    sc = scale_sb[:, o // 128 : (o + G) // 128]
                    nc.vector.tensor_tensor(
                        out=g[:],
                        in0=g[:],
                        in1=sc[:, :, None].to_broadcast([128, G // 128, D]),
                        op=mybir.AluOpType.mult,
                    )
                    # one scatter sub-call per duplicate-rank span piece:
                    # destination indices are unique within each call
                    so = 0
                    for span in spans[u]:
                        for po in range(0, span, MAXC):
                            n = min(MAXC, span - po)
                            p0 = so + po
                            nc.gpsimd.dma_scatter_add(
                                out_ap=A[lay][rpi].ap()[:, :],
                                in_ap=g[:, p0 // 128 : (p0 + n) // 128, :],
                                idxs_ap=sidx_sb[
                                    :, (o + p0) // 16 : (o + p0 + n) // 16
                                ],
                                num_idxs=n,
                                num_idxs_reg=n,
                                elem_size=D,
                                single_packet=SINGLE_PACKET,
                            )
                        so += span

                nc.leave_named_scope(f"edge_{lay}", es[0], False)
                ts = nc.enter_named_scope(f"transform_{lay}", False)
                # ---- weights for this layer ----
                w_sb = wpool.tile([128, (R + 1) * D], BF16, tag="w", bufs=2)
                nc.sync.dma_start(
                    out=w_sb[:].rearrange("d (r e) -> d r e", r=R + 1),
                    in_=w_all.ap()[lay].rearrange("r d e -> d r e"),
                )
                b_sb = wpool.tile([1, D], BF16, tag="b", bufs=2)
                nc.sync.dma_start(out=b_sb[:], in_=b_all.ap()[lay])

                # ---- transform phase ----
                for ch in range(NCH):
                    row0 = ch * TCH * 128
                    sts = []
                    for r in range(R):
                        st = spool.tile([128, TCH * 128], BF16, tag=f"st{r}")
                        a_rows = A[lay][r // 2].ap()[
                            (r % 2) * NLP + row0 : (r % 2) * NLP + row0 + TCH * 128, :
                        ]
                        nc.sync.dma_start_transpose(out=st[:], in_=a_rows)
                        sts.append(st)
                    st_x = spool.tile([128, TCH * 128], BF16, tag="stx")
                    nc.sync.dma_start_transpose(
                        out=st_x[:], in_=loc_tab.ap()[row0 : row0 + TCH * 128, :]
                    )

                    for t in range(TCH):
                        ps = psum_pool.tile([128, D], F32, tag="ps")
                        for r in range(R):
                            nc.tensor.matmul(
                                out=ps[:],
                                lhsT=sts[r][:, t * 128 : (t + 1) * 128],
                                rhs=w_sb[:, r * D : (r + 1) * D],
                                start=(r == 0),
                                stop=False,
                            )
                        nc.tensor.matmul(
                            out=ps[:],
                            lhsT=st_x[:, t * 128 : (t + 1) * 128],
                            rhs=w_sb[:, R * D : (R + 1) * D],
                            start=False,
                            stop=False,
                        )
                        nc.tensor.matmul(
                            out=ps[:],
                            lhsT=ones_sb[:1, :],
                            rhs=b_sb[:1, :],
                            start=False,
                            stop=True,
                        )
                        row = row0 + t * 128
                        if lay == 0:
                            hs = hpool.tile([128, D], BF16, tag="h0")
                            nc.scalar.activation(
                                out=hs[:], in_=ps[:],
                                func=mybir.ActivationFunctionType.Relu,
                            )
                            nc.sync.dma_start(
                                out=h1b.ap()[row : row + 128, :], in_=hs[:]
                            )
                        else:
                            nrow = min(128, NL - row)
                            if nrow <= 0:
                                continue
                            hs = hpool.tile([128, D], F32, tag="h1")
                            nc.scalar.activation(
                                out=hs[:], in_=ps[:],
                                func=mybir.ActivationFunctionType.Relu,
                            )
                            nc.sync.dma_start(
                                out=out_d.ap()[row : row + nrow, :],
                                in_=hs[:nrow, :],
                            )

                nc.leave_named_scope(f"transform_{lay}", ts[0], False)
                # ---- inter-layer AllGather (replicate new features) ----
                if lay == 0:
                    nc.gpsimd.collective_compute(
                        "AllGather",
                        mybir.AluOpType.bypass,
                        replica_groups=[list(range(C))],
                        ins=[h1b.ap()],
                        outs=[h1rep.ap()],
                    )

    nc.compile()
    return nc


# ----------------------------------------------------------------------------
# In-map assembly (shared by kernel() and tests)
# ----------------------------------------------------------------------------
def make_in_maps(prep, W1, root1, b1, W2, root2, b2, cfg):
    C, D, R = cfg["C"], cfg["D"], cfg["R"]
    NL, NT, NLP, RP, TCH = derive(cfg)
    w_all = np.zeros((2, R + 1, D, D), bf16)
    w_all[0, :R] = np.asarray(W1, np.float32).astype(bf16)
    w_all[0, R] = np.asarray(root1, np.float32).astype(bf16)
    w_all[1, :R] = np.asarray(W2, np.float32).astype(bf16)
    w_all[1, R] = np.asarray(root2, np.float32).astype(bf16)
    b_stack = np.stack([np.asarray(b1, np.float32), np.asarray(b2, np.float32)])
    b_all = b_stack.reshape(2, 1, D).astype(bf16)

    in_maps = []
    for c in range(C):
        x_loc = np.ascontiguousarray(prep["x_rep"][c * NLP : (c + 1) * NLP])
        in_maps.append({
            "x_rep": prep["x_rep"],
            "x_loc": x_loc,
            "w_all": w_all,
            "b_all": b_all,
            "gidx": prep["gidx"][c],
            "sidx": prep["sidx"][c],
            "scale": prep["scale"][c],
        })
    return in_maps


def enable_ntff_hook():
    """Register the axon NTFF profiling hook if the image's antenv lacks it."""
    import sys, types
    try:
        import antenv.axon_hooks  # noqa: F401
        return True
    except ImportError:
        pass
    try:
        from trn_agent_boot.trn_boot import _ntff_profile_via_ctypes
        hook = _ntff_profile_via_ctypes("/opt/axon/libaxon_pjrt.so")
        mod = types.ModuleType("antenv.axon_hooks")
        mod._hook = hook
        mod.set_axon_ntff_profile_hook = lambda h: setattr(mod, "_hook", h)
        mod.get_axon_ntff_profile_hook = lambda: mod._hook
        sys.modules["antenv.axon_hooks"] = mod
        import antenv
        antenv.axon_hooks = mod
        return hook is not None
    except Exception:
        return False


_program_cache = {}


def run(x, edge_index, edge_type, W1, root1, b1, W2, root2, b2,
        cfg=FULL, trace=False):
    prep = host_prep(x, edge_index, edge_type, cfg)
    key = (tuple(sorted(cfg.items())), prep["spans"], prep["NU"])
    if key not in _program_cache:
        _program_cache[key] = build_program(cfg, prep["spans"], prep["NU"])
    nc = _program_cache[key]
    in_maps = make_in_maps(prep, W1, root1, b1, W2, root2, b2, cfg)
    if trace:
        trace = enable_ntff_hook()
    res = run_bass_kernel_spmd(
        nc, in_maps, core_ids=list(range(cfg["C"])), trace=trace
    )
    blocks = [res.results[c]["out"] for c in range(cfg["C"])]
    full = np.concatenate(blocks, axis=0).astype(np.float32)
    return full, res


def kernel(**inputs):
    out, _ = run(
        inputs["x"], inputs["edge_index"], inputs["edge_type"],
        inputs["W1"], inputs["root1"], inputs["b1"],
        inputs["W2"], inputs["root2"], inputs["b2"],
    )
    return out



# revision 13
# speedup vs baseline: 1.4775x; 1.4775x over previous
"""RGCN (2-layer, mean-aggregation) Bass kernel for one TRN2 chip (8 NeuronCores).

Strategy (dst-sharded, matmul-based aggregation — no DRAM scatter):
  - Nodes block-partitioned across 8 cores (12500/core, padded to 12544).
    Edges live on their dst-owner core; x is replicated (bf16, padded layout
    [C*NLP, D]) in every core's HBM.
  - Edges are sorted by (dst-tile-batch, src-quarter, rel-pair, tile) and
    padded per group to 128. Per (batch, src-quarter) run: dma_gather the
    messages x[src] ([128e, D] natural layout), scale by inv_deg (DVE), build
    a one-hot matrix M[e, (rel%2)*128 + dst%128] with a DVE is_equal against
    a resident iota row, then aggregate on the tensor engine:
        A_pairT[d, code] += msg^T @ M   (PSUM, f32 accumulation)
    A_pairT is directly the lhsT for the transform matmuls:
        out[dst, :] = relu(sum_r A_rT.T @ W_r + x_locT.T @ Wroot + b)
  - The dst one-hot lives in rhs (free dim 256 = rel-parity x dst128), PSUM
    holds 4 rel-pair accumulators per 2-tile batch (4 banks).
  - Gathers use int16 indices over 4 equal row-quarters of the replicated
    table (25088 rows < 32768), one SWDGE queue per quarter.
  - Between layers one small AllGather (bf16, 3.2MB/rank) replicates the new
    features; layer 1's replication is free (host pre-stages x_rep).
"""

import os
import numpy as np
import ml_dtypes

import concourse.tile as tile
from concourse import bass, bacc, mybir
from concourse.bass_utils import run_bass_kernel_spmd

BF16 = mybir.dt.bfloat16
F32 = mybir.dt.float32
I16 = mybir.dt.int16
bf16 = ml_dtypes.bfloat16

K_QUEUES = int(os.environ.get("K_QUEUES", "4"))
SINGLE_PACKET = os.environ.get("K_SP", "0") == "1"
MAXC = int(os.environ.get("K_MAXC", "1024" if SINGLE_PACKET else "1920"))
# M source: 0 = build one-hot on DVE (is_equal), 1 = DMA precomputed fp8
K_MDRAM = int(os.environ.get("K_MDRAM", "0"))
FP8 = mybir.dt.float8e4
fp8 = ml_dtypes.float8_e4m3fn

# ----------------------------------------------------------------------------
# Problem constants
# ----------------------------------------------------------------------------
FULL = dict(N=100000, E=1000000, D=128, R=8, C=8)


def derive(cfg):
    N, D, R, C = cfg["N"], cfg["D"], cfg["R"], cfg["C"]
    NL = N // C                      # owned nodes per core
    NT = (NL + 127) // 128           # dst tiles per core
    NLP = NT * 128                   # padded rows per block
    NTOT = C * NLP                   # replicated-table rows
    B = 1                            # dst tiles per batch (PSUM bank math)
    NBAT = NT // B                   # batches per core
    PAIRS = R // 2
    NSB = 4                          # src table quarters
    SBR = NTOT // NSB                # rows per quarter (must be < 32768)
    MW = 256                         # M width: (rel%2)*128 + dst%128
    return NL, NT, NLP, NTOT, B, NBAT, PAIRS, NSB, SBR, MW


# ----------------------------------------------------------------------------
# Host-side preprocessing
# ----------------------------------------------------------------------------
def host_prep(x, edge_index, edge_type, cfg):
    N, E, D, R, C = cfg["N"], cfg["E"], cfg["D"], cfg["R"], cfg["C"]
    NL, NT, NLP, NTOT, B, NBAT, PAIRS, NSB, SBR, MW = derive(cfg)
    assert SBR < 32768 and NT % B == 0

    src = np.asarray(edge_index[0], dtype=np.int64)
    dst = np.asarray(edge_index[1], dtype=np.int64)
    et = np.asarray(edge_type, dtype=np.int64)

    # mean-normalization per (relation, dst), computed on host (graph-only)
    deg = np.zeros((R, N), np.float32)
    np.add.at(deg, (et, dst), 1.0)
    inv = np.where(deg > 0, 1.0 / np.maximum(deg, 1.0), 0.0).astype(np.float32)
    scale_e = inv[et, dst]

    core = dst // NL
    dl = dst % NL
    tl = dl // 128
    bat = tl // B
    t2 = tl % B
    pair = et // 2
    code = (et % 2) * 128 + (dl % 128)
    srcp = (src // NL) * NLP + (src % NL)       # padded replicated-table row
    sb = srcp // SBR
    sidx = (srcp % SBR).astype(np.int64)

    NG = NBAT * NSB * PAIRS * B
    g = ((bat * NSB + sb) * PAIRS + pair) * B + t2

    counts = np.zeros((C, NG), np.int64)
    np.add.at(counts, (core, g), 1)
    gsz = np.maximum(((counts.max(axis=0) + 127) // 128) * 128, 128)  # [NG]
    offs = np.zeros(NG + 1, np.int64)
    np.cumsum(gsz, out=offs[1:])
    PAD = int(offs[-1])

    # place edges: stable sort by (core, g), rank within each (core, g) run
    key = core * NG + g
    order = np.argsort(key, kind="stable")
    key_o = key[order]
    new_run = np.ones(E, bool)
    new_run[1:] = key_o[1:] != key_o[:-1]
    run_starts = np.flatnonzero(new_run)
    run_id = np.cumsum(new_run) - 1
    rank = np.arange(E) - run_starts[run_id]
    pos = offs[g[order]] + rank

    gidx_a = np.zeros((C, PAD), np.int16)
    code_a = np.full((C, PAD), 300.0, bf16)
    scal_a = np.zeros((C, PAD), bf16)
    co = core[order]
    gidx_a[co, pos] = sidx[order].astype(np.int16)
    code_a[co, pos] = code[order].astype(bf16)
    scal_a[co, pos] = scale_e[order].astype(bf16)

    # wrapped layouts: idx i at [i%16, i//16] (x8 down); code/scale at
    # [i%128, i//128]
    gidx_w = np.tile(
        gidx_a.reshape(C, PAD // 16, 16).transpose(0, 2, 1), (1, 8, 1)
    )
    dstv_w = np.ascontiguousarray(
        code_a.reshape(C, PAD // 128, 128).transpose(0, 2, 1)
    )
    scal_w = np.ascontiguousarray(
        scal_a.reshape(C, PAD // 128, 128).transpose(0, 2, 1)
    )

    # precomputed one-hot M (fp8), wrapped [C, 128, (PAD//128)*MW]
    m_w = None
    if K_MDRAM:
        codes = np.asarray(code_a, np.float32).astype(np.int32)
        m_full = (
            codes.reshape(C, PAD // 128, 128)[..., None]
            == np.arange(MW, dtype=np.int32)
        )
        m_w = np.ascontiguousarray(
            m_full.transpose(0, 2, 1, 3).reshape(C, 128, (PAD // 128) * MW)
        ).astype(fp8)

    # replicated, block-padded x (bf16): [NTOT, D]
    x = np.asarray(x, np.float32)
    x_rep = np.zeros((NTOT, D), bf16)
    for c in range(C):
        x_rep[c * NLP : c * NLP + NL] = x[c * NL : (c + 1) * NL].astype(bf16)

    # run table: per (bat, sb): (offset, size); groups within are (pair, t2)
    runs = []
    for b_ in range(NBAT):
        row = []
        for s_ in range(NSB):
            g0 = ((b_ * NSB + s_) * PAIRS) * B
            o = int(offs[g0])
            n = int(offs[g0 + PAIRS * B] - offs[g0])
            row.append((o, n))
        runs.append(row)

    return dict(
        gsz=tuple(int(v) for v in gsz),
        runs=tuple(tuple(r) for r in runs),
        PAD=PAD,
        gidx=np.ascontiguousarray(gidx_w),
        dstv=dstv_w,
        scal=scal_w,
        m_w=m_w,
        x_rep=x_rep,
    )


# ----------------------------------------------------------------------------
# Device program
# ----------------------------------------------------------------------------
def build_program(cfg, gsz, runs, PAD):
    N, E, D, R, C = cfg["N"], cfg["E"], cfg["D"], cfg["R"], cfg["C"]
    NL, NT, NLP, NTOT, B, NBAT, PAIRS, NSB, SBR, MW = derive(cfg)

    nc = bacc.Bacc(
        "TRN2", target_bir_lowering=False, debug=False,
        enable_asserts=False, num_devices=C, num_swdge_queues=K_QUEUES,
    )

    x_rep = nc.dram_tensor("x_rep", [NTOT, D], BF16, kind="ExternalInput")
    x_loc = nc.dram_tensor("x_loc", [NLP, D], BF16, kind="ExternalInput")
    w_all = nc.dram_tensor("w_all", [2, R + 1, D, D], BF16, kind="ExternalInput")
    b_all = nc.dram_tensor("b_all", [2, 1, D], BF16, kind="ExternalInput")
    gidx_d = nc.dram_tensor("gidx", [128, PAD // 16], I16, kind="ExternalInput")
    dstv_d = nc.dram_tensor("dstv", [128, PAD // 128], BF16, kind="ExternalInput")
    scal_d = nc.dram_tensor("scal", [128, PAD // 128], BF16, kind="ExternalInput")
    ciota_d = nc.dram_tensor("ciota", [128, MW], BF16, kind="ExternalInput")
    if K_MDRAM:
        m_d = nc.dram_tensor(
            "m_w", [128, (PAD // 128) * MW], FP8, kind="ExternalInput"
        )
    out_d = nc.dram_tensor("out", [NL, D], F32, kind="ExternalOutput")
    h1b = nc.dram_tensor("h1b", [NLP, D], BF16, kind="Internal")
    h1rep = nc.dram_tensor(
        "h1rep", [NTOT, D], BF16, kind="Internal", addr_space="Shared"
    )

    with tile.TileContext(nc) as tc:
        with (
            tc.tile_pool(name="resident", bufs=1) as res_pool,
            tc.tile_pool(name="msg", bufs=3) as msg_pool,
            tc.tile_pool(name="mm", bufs=3) as m_pool,
            tc.tile_pool(name="asb", bufs=2) as a_pool,
            tc.tile_pool(name="loct", bufs=2) as loct_pool,
            tc.tile_pool(name="wpool", bufs=1) as wpool,
            tc.tile_pool(name="hout", bufs=4) as hpool,
            tc.tile_pool(name="psA", bufs=1, space="PSUM") as psA_pool,
        ):
            gidx_sb = res_pool.tile([128, PAD // 16], I16)
            dstv_sb = res_pool.tile([128, PAD // 128], BF16)
            scal_sb = res_pool.tile([128, PAD // 128], BF16)
            ciota_sb = res_pool.tile([128, MW], BF16)
            nc.sync.dma_start(out=gidx_sb[:], in_=gidx_d.ap()[:, :])
            nc.sync.dma_start(out=dstv_sb[:], in_=dstv_d.ap()[:, :])
            nc.sync.dma_start(out=scal_sb[:], in_=scal_d.ap()[:, :])
            nc.sync.dma_start(out=ciota_sb[:], in_=ciota_d.ap()[:, :])
            ones_sb = res_pool.tile([1, D], BF16)
            nc.vector.memset(ones_sb[:], 1.0)

            for lay in range(2):
                src_tab = x_rep if lay == 0 else h1rep
                loc_tab = x_loc if lay == 0 else h1b

                ls = nc.enter_named_scope(f"lay_{lay}", False)
                w_sb = wpool.tile([128, (R + 1) * D], BF16, tag="w", bufs=2)
                nc.sync.dma_start(
                    out=w_sb[:].rearrange("d (r e) -> d r e", r=R + 1),
                    in_=w_all.ap()[lay].rearrange("r d e -> d r e"),
                )
                b_sb = wpool.tile([1, D], BF16, tag="b", bufs=2)
                nc.sync.dma_start(out=b_sb[:], in_=b_all.ap()[lay])

                for bat in range(NBAT):
                    row0 = bat * 128
                    if bat % 2 == 0:
                        nrows = min(256, NLP - row0)
                        loct = loct_pool.tile([128, 256], BF16, tag="lt")
                        nc.sync.dma_start_transpose(
                            out=loct[:, :nrows],
                            in_=loc_tab.ap()[row0 : row0 + nrows, :],
                        )
                    # 4 pair-accumulators, each padded to a full PSUM bank so
                    # every concurrently-open accumulation group owns its own
                    # 2KB zero region (PE start=True zeroes the whole region)
                    psA = [
                        psA_pool.tile([128, MW], F32, tag=f"pa{p}",
                                      name=f"psA{p}", bufs=2,
                                      padded_shape=[128, 512])
                        for p in range(PAIRS)
                    ]
                    for sb in range(NSB):
                        o, n = runs[bat][sb]
                        nch = n // 128
                        msgt = msg_pool.tile([128, nch, D], BF16, tag="msg")
                        for co in range(0, n, MAXC):
                            cn = min(MAXC, n - co)
                            nc.gpsimd.dma_gather(
                                out_ap=msgt[:, co // 128 : (co + cn) // 128, :],
                                in_ap=src_tab.ap()[sb * SBR : (sb + 1) * SBR, :],
                                idxs_ap=gidx_sb[
                                    :, (o + co) // 16 : (o + co + cn) // 16
                                ],
                                num_idxs=cn,
                                num_idxs_reg=cn,
                                elem_size=D,
                                single_packet=SINGLE_PACKET,
                                queue_num=sb % K_QUEUES,
                            )
                        nc.vector.tensor_tensor(
                            out=msgt[:],
                            in0=msgt[:],
                            in1=scal_sb[:, o // 128 : (o + n) // 128, None]
                            .to_broadcast([128, nch, D]),
                            op=mybir.AluOpType.mult,
                        )
                        if K_MDRAM:
                            mt = m_pool.tile([128, nch * MW], FP8, tag="m")
                            nc.sync.dma_start(
                                out=mt[:],
                                in_=m_d.ap()[
                                    :, (o // 128) * MW : ((o + n) // 128) * MW
                                ],
                            )
                        else:
                            mt = m_pool.tile([128, nch * MW], BF16, tag="m")
                            nc.vector.tensor_tensor(
                                out=mt[:].rearrange(
                                    "p (a b) -> p a b", b=MW
                                ),
                                in0=dstv_sb[:, o // 128 : (o + n) // 128, None]
                                .to_broadcast([128, nch, MW]),
                                in1=ciota_sb[:, None, :]
                                .to_broadcast([128, nch, MW]),
                                op=mybir.AluOpType.is_equal,
                            )
                        ci = 0
                        for p in range(PAIRS):
                            gi = (bat * NSB + sb) * PAIRS + p
                            gch = gsz[gi] // 128
                            for k in range(gch):
                                nc.tensor.matmul(
                                    out=psA[p][:],
                                    lhsT=msgt[:, ci, :],
                                    rhs=mt[:, ci * MW : (ci + 1) * MW],
                                    start=(sb == 0 and k == 0),
                                    stop=(sb == NSB - 1 and k == gch - 1),
                                )
                                ci += 1
                        assert ci == nch

                    a_sb = [
                        a_pool.tile([128, MW], BF16, tag=f"a{p}",
                                    name=f"a_sb{p}", bufs=2)
                        for p in range(PAIRS)
                    ]
                    for p in range(PAIRS):
                        nc.scalar.activation(
                            out=a_sb[p][:], in_=psA[p][:],
                            func=mybir.ActivationFunctionType.Copy,
                        )

                    # transform reuses psA[3]'s bank (its group is closed and
                    # its data copied to SBUF by now)
                    ps = psA[3][:, 0:D]
                    for r in range(R):
                        nc.tensor.matmul(
                            out=ps,
                            lhsT=a_sb[r // 2][:, (r % 2) * 128 : (r % 2) * 128 + 128],
                            rhs=w_sb[:, r * D : (r + 1) * D],
                            start=(r == 0),
                            stop=False,
                        )
                    nc.tensor.matmul(
                        out=ps,
                        lhsT=loct[:, (bat % 2) * 128 : (bat % 2) * 128 + 128],
                        rhs=w_sb[:, R * D : (R + 1) * D],
                        start=False,
                        stop=False,
                    )
                    nc.tensor.matmul(
                        out=ps,
                        lhsT=ones_sb[:1, :],
                        rhs=b_sb[:1, :],
                        start=False,
                        stop=True,
                    )
                    if lay == 0:
                        hs = hpool.tile([128, D], BF16, tag="h0")
                        nc.scalar.activation(
                            out=hs[:], in_=ps,
                            func=mybir.ActivationFunctionType.Relu,
                        )
                        nc.sync.dma_start(
                            out=h1b.ap()[row0 : row0 + 128, :], in_=hs[:]
                        )
                    else:
                        nrow = min(128, NL - row0)
                        if nrow <= 0:
                            continue
                        hs = hpool.tile([128, D], F32, tag="h1")
                        nc.scalar.activation(
                            out=hs[:], in_=ps,
                            func=mybir.ActivationFunctionType.Relu,
                        )
                        nc.sync.dma_start(
                            out=out_d.ap()[row0 : row0 + nrow, :],
                            in_=hs[:nrow, :],
                        )

                nc.leave_named_scope(f"lay_{lay}", ls[0], False)
                if lay == 0:
                    nc.gpsimd.collective_compute(
                        "AllGather",
                        mybir.AluOpType.bypass,
                        replica_groups=[list(range(C))],
                        ins=[h1b.ap()],
                        outs=[h1rep.ap()],
                    )

    nc.compile()
    return nc


# ----------------------------------------------------------------------------
# In-map assembly
# ----------------------------------------------------------------------------
def make_in_maps(prep, W1, root1, b1, W2, root2, b2, cfg):
    C, D, R = cfg["C"], cfg["D"], cfg["R"]
    NL, NT, NLP, NTOT, B, NBAT, PAIRS, NSB, SBR, MW = derive(cfg)
    w_all = np.zeros((2, R + 1, D, D), bf16)
    w_all[0, :R] = np.asarray(W1, np.float32).astype(bf16)
    w_all[0, R] = np.asarray(root1, np.float32).astype(bf16)
    w_all[1, :R] = np.asarray(W2, np.float32).astype(bf16)
    w_all[1, R] = np.asarray(root2, np.float32).astype(bf16)
    b_stack = np.stack([np.asarray(b1, np.float32), np.asarray(b2, np.float32)])
    b_all = b_stack.reshape(2, 1, D).astype(bf16)
    ciota = np.tile(np.arange(MW, dtype=np.float32).astype(bf16), (128, 1))

    in_maps = []
    for c in range(C):
        x_loc = np.ascontiguousarray(prep["x_rep"][c * NLP : (c + 1) * NLP])
        im = {
            "x_rep": prep["x_rep"],
            "x_loc": x_loc,
            "w_all": w_all,
            "b_all": b_all,
            "gidx": prep["gidx"][c],
            "dstv": prep["dstv"][c],
            "scal": prep["scal"][c],
            "ciota": ciota,
        }
        if K_MDRAM:
            im["m_w"] = prep["m_w"][c]
        in_maps.append(im)
    return in_maps


def enable_ntff_hook():
    """Register the axon NTFF profiling hook if the image's antenv lacks it."""
    import sys, types
    try:
        import antenv.axon_hooks  # noqa: F401
        return True
    except ImportError:
        pass
    try:
        from trn_agent_boot.trn_boot import _ntff_profile_via_ctypes
        hook = _ntff_profile_via_ctypes("/opt/axon/libaxon_pjrt.so")
        mod = types.ModuleType("antenv.axon_hooks")
        mod._hook = hook
        mod.set_axon_ntff_profile_hook = lambda h: setattr(mod, "_hook", h)
        mod.get_axon_ntff_profile_hook = lambda: mod._hook
        sys.modules["antenv.axon_hooks"] = mod
        import antenv
        antenv.axon_hooks = mod
        return hook is not None
    except Exception:
        return False


_program_cache = {}


def run(x, edge_index, edge_type, W1, root1, b1, W2, root2, b2,
        cfg=FULL, trace=False):
    prep = host_prep(x, edge_index, edge_type, cfg)
    key = (tuple(sorted(cfg.items())), prep["gsz"], prep["runs"], prep["PAD"],
           K_QUEUES, SINGLE_PACKET, MAXC, K_MDRAM)
    if key not in _program_cache:
        _program_cache[key] = build_program(
            cfg, prep["gsz"], prep["runs"], prep["PAD"]
        )
    nc = _program_cache[key]
    in_maps = make_in_maps(prep, W1, root1, b1, W2, root2, b2, cfg)
    if trace:
        trace = enable_ntff_hook()
    res = run_bass_kernel_spmd(
        nc, in_maps, core_ids=list(range(cfg["C"])), trace=trace
    )
    blocks = [res.results[c]["out"] for c in range(cfg["C"])]
    full = np.concatenate(blocks, axis=0).astype(np.float32)
    return full, res


def kernel(**inputs):
    out, _ = run(
        inputs["x"], inputs["edge_index"], inputs["edge_type"],
        inputs["W1"], inputs["root1"], inputs["b1"],
        inputs["W2"], inputs["root2"], inputs["b2"],
    )
    return out


# revision 23
# speedup vs baseline: 2.2555x; 1.5266x over previous
"""RGCN (2-layer, mean-aggregation) Bass kernel for one TRN2 chip (8 NeuronCores).

Strategy (dst-sharded, matmul-based aggregation — no DRAM scatter):
  - Nodes block-partitioned across 8 cores (12500/core, padded to 12544).
    Edges live on their dst-owner core; x is replicated (bf16, padded layout
    [C*NLP, D]) in every core's HBM.
  - Edges are sorted by (dst-tile-batch, src-quarter, rel-pair, tile) and
    padded per group to 128. Per (batch, src-quarter) run: dma_gather the
    messages x[src] ([128e, D] natural layout), scale by inv_deg (DVE), build
    a one-hot matrix M[e, (rel%2)*128 + dst%128] with a DVE is_equal against
    a resident iota row, then aggregate on the tensor engine:
        A_pairT[d, code] += msg^T @ M   (PSUM, f32 accumulation)
    A_pairT is directly the lhsT for the transform matmuls:
        out[dst, :] = relu(sum_r A_rT.T @ W_r + x_locT.T @ Wroot + b)
  - The dst one-hot lives in rhs (free dim 256 = rel-parity x dst128), PSUM
    holds 4 rel-pair accumulators per 2-tile batch (4 banks).
  - Gathers use int16 indices over 4 equal row-quarters of the replicated
    table (25088 rows < 32768), one SWDGE queue per quarter.
  - Between layers one small AllGather (bf16, 3.2MB/rank) replicates the new
    features; layer 1's replication is free (host pre-stages x_rep).
"""

import os
import numpy as np
import ml_dtypes

import concourse.tile as tile
from concourse import bass, bacc, mybir
from concourse.bass_utils import run_bass_kernel_spmd

BF16 = mybir.dt.bfloat16
F32 = mybir.dt.float32
I16 = mybir.dt.int16
bf16 = ml_dtypes.bfloat16

K_QUEUES = int(os.environ.get("K_QUEUES", "4"))
SINGLE_PACKET = os.environ.get("K_SP", "0") == "1"
MAXC = int(os.environ.get("K_MAXC", "1024" if SINGLE_PACKET else "1920"))
# M source: 0 = build one-hot on DVE (is_equal), 1 = DMA precomputed fp8
K_MDRAM = int(os.environ.get("K_MDRAM", "1"))
K_GRP = int(os.environ.get("K_GRP", "2"))   # relations per accumulator group
K_B = int(os.environ.get("K_B", "1"))       # dst tiles per batch
FP8 = mybir.dt.float8e4
fp8 = ml_dtypes.float8_e4m3fn

# ----------------------------------------------------------------------------
# Problem constants
# ----------------------------------------------------------------------------
FULL = dict(N=100000, E=1000000, D=128, R=8, C=8)


def derive(cfg):
    N, D, R, C = cfg["N"], cfg["D"], cfg["R"], cfg["C"]
    NL = N // C                      # owned nodes per core
    NT = (NL + 127) // 128           # dst tiles per core
    NLP = NT * 128                   # padded rows per block
    NTOT = C * NLP                   # replicated-table rows
    B = K_B                          # dst tiles per batch
    NBAT = NT // B                   # batches per core
    PAIRS = R // K_GRP               # accumulator groups per (tile)
    NSB = 4                          # src table quarters
    SBR = NTOT // NSB                # rows per quarter (must be < 32768)
    MW = K_GRP * 128                 # M width: (rel%GRP)*128 + dst%128
    # PSUM: PAIRS*B accumulators, each padded to a full 2KB bank, x2 bufs
    assert PAIRS * B * 2 <= 8 and MW <= 512
    return NL, NT, NLP, NTOT, B, NBAT, PAIRS, NSB, SBR, MW


# ----------------------------------------------------------------------------
# Host-side preprocessing
# ----------------------------------------------------------------------------
def host_prep(x, edge_index, edge_type, cfg):
    N, E, D, R, C = cfg["N"], cfg["E"], cfg["D"], cfg["R"], cfg["C"]
    NL, NT, NLP, NTOT, B, NBAT, PAIRS, NSB, SBR, MW = derive(cfg)
    assert SBR < 32768 and NT % B == 0

    src = np.asarray(edge_index[0], dtype=np.int64)
    dst = np.asarray(edge_index[1], dtype=np.int64)
    et = np.asarray(edge_type, dtype=np.int64)

    # mean-normalization per (relation, dst), computed on host (graph-only)
    deg = np.zeros((R, N), np.float32)
    np.add.at(deg, (et, dst), 1.0)
    inv = np.where(deg > 0, 1.0 / np.maximum(deg, 1.0), 0.0).astype(np.float32)
    scale_e = inv[et, dst]

    core = dst // NL
    dl = dst % NL
    tl = dl // 128
    bat = tl // B
    t2 = tl % B
    pair = et // K_GRP
    code = (et % K_GRP) * 128 + (dl % 128)
    srcp = (src // NL) * NLP + (src % NL)       # padded replicated-table row
    sb = srcp // SBR
    sidx = (srcp % SBR).astype(np.int64)

    NG = NBAT * NSB * PAIRS * B
    g = ((bat * NSB + sb) * PAIRS + pair) * B + t2

    counts = np.zeros((C, NG), np.int64)
    np.add.at(counts, (core, g), 1)
    gsz = np.maximum(((counts.max(axis=0) + 127) // 128) * 128, 128)  # [NG]
    offs = np.zeros(NG + 1, np.int64)
    np.cumsum(gsz, out=offs[1:])
    PAD = int(offs[-1])

    # place edges: stable sort by (core, g), rank within each (core, g) run
    key = core * NG + g
    order = np.argsort(key, kind="stable")
    key_o = key[order]
    new_run = np.ones(E, bool)
    new_run[1:] = key_o[1:] != key_o[:-1]
    run_starts = np.flatnonzero(new_run)
    run_id = np.cumsum(new_run) - 1
    rank = np.arange(E) - run_starts[run_id]
    pos = offs[g[order]] + rank

    gidx_a = np.zeros((C, PAD), np.int16)
    code_i = np.full((C, PAD), 10000, np.int32)   # pad sentinel, out of range
    scal_a = np.zeros((C, PAD), bf16)
    co = core[order]
    gidx_a[co, pos] = sidx[order].astype(np.int16)
    code_i[co, pos] = code[order].astype(np.int32)
    scal_a[co, pos] = scale_e[order].astype(bf16)
    # bf16 codes only feed the K_MDRAM=0 is_equal path; bf16 integers are
    # exact to 256 so that path requires MW <= 256
    assert K_MDRAM or MW <= 256
    code_a = np.minimum(code_i, 300).astype(bf16)

    # wrapped layouts: idx i at [i%16, i//16] (x8 down); code/scale at
    # [i%128, i//128]
    gidx_w = np.tile(
        gidx_a.reshape(C, PAD // 16, 16).transpose(0, 2, 1), (1, 8, 1)
    )
    dstv_w = np.ascontiguousarray(
        code_a.reshape(C, PAD // 128, 128).transpose(0, 2, 1)
    )
    scal_w = np.ascontiguousarray(
        scal_a.reshape(C, PAD // 128, 128).transpose(0, 2, 1)
    )

    # precomputed one-hot M (fp8), wrapped [C, 128, (PAD//128)*MW]
    m_w = None
    if K_MDRAM:
        m_full = (
            code_i.reshape(C, PAD // 128, 128)[..., None]
            == np.arange(MW, dtype=np.int32)
        )
        m_w = np.ascontiguousarray(
            m_full.transpose(0, 2, 1, 3).reshape(C, 128, (PAD // 128) * MW)
        ).astype(fp8)

    # replicated, block-padded x (bf16): [NTOT, D]
    x = np.asarray(x, np.float32)
    x_rep = np.zeros((NTOT, D), bf16)
    for c in range(C):
        x_rep[c * NLP : c * NLP + NL] = x[c * NL : (c + 1) * NL].astype(bf16)

    # run table: per (bat, sb): (offset, size); groups within are (pair, t2)
    runs = []
    for b_ in range(NBAT):
        row = []
        for s_ in range(NSB):
            g0 = ((b_ * NSB + s_) * PAIRS) * B
            o = int(offs[g0])
            n = int(offs[g0 + PAIRS * B] - offs[g0])
            row.append((o, n))
        runs.append(row)

    return dict(
        gsz=tuple(int(v) for v in gsz),
        runs=tuple(tuple(r) for r in runs),
        PAD=PAD,
        gidx=np.ascontiguousarray(gidx_w),
        dstv=dstv_w,
        scal=scal_w,
        m_w=m_w,
        code_i=code_i,
        x_rep=x_rep,
    )


# ----------------------------------------------------------------------------
# Device program
# ----------------------------------------------------------------------------
def build_program(cfg, gsz, runs, PAD):
    N, E, D, R, C = cfg["N"], cfg["E"], cfg["D"], cfg["R"], cfg["C"]
    NL, NT, NLP, NTOT, B, NBAT, PAIRS, NSB, SBR, MW = derive(cfg)

    nc = bacc.Bacc(
        "TRN2", target_bir_lowering=False, debug=False,
        enable_asserts=False, num_devices=C, num_swdge_queues=K_QUEUES,
    )

    x_rep = nc.dram_tensor("x_rep", [NTOT, D], BF16, kind="ExternalInput")
    x_loc = nc.dram_tensor("x_loc", [NLP, D], BF16, kind="ExternalInput")
    w_all = nc.dram_tensor("w_all", [2, R + 1, D, D], BF16, kind="ExternalInput")
    b_all = nc.dram_tensor("b_all", [2, 1, D], BF16, kind="ExternalInput")
    gidx_d = nc.dram_tensor("gidx", [128, PAD // 16], I16, kind="ExternalInput")
    dstv_d = nc.dram_tensor("dstv", [128, PAD // 128], BF16, kind="ExternalInput")
    scal_d = nc.dram_tensor("scal", [128, PAD // 128], BF16, kind="ExternalInput")
    ciota_d = nc.dram_tensor("ciota", [128, MW], BF16, kind="ExternalInput")
    if K_MDRAM:
        m_d = nc.dram_tensor(
            "m_w", [128, (PAD // 128) * MW], FP8, kind="ExternalInput"
        )
    out_d = nc.dram_tensor("out", [NL, D], F32, kind="ExternalOutput")
    h1b = nc.dram_tensor("h1b", [NLP, D], BF16, kind="Internal")
    h1rep = nc.dram_tensor(
        "h1rep", [NTOT, D], BF16, kind="Internal", addr_space="Shared"
    )

    with tile.TileContext(nc) as tc:
        with (
            tc.tile_pool(name="resident", bufs=1) as res_pool,
            tc.tile_pool(name="msg", bufs=8) as msg_pool,
            tc.tile_pool(name="mm", bufs=8) as m_pool,
            tc.tile_pool(name="asb", bufs=2) as a_pool,
            tc.tile_pool(name="loct", bufs=2) as loct_pool,
            tc.tile_pool(name="wpool", bufs=1) as wpool,
            tc.tile_pool(name="hout", bufs=4) as hpool,
            tc.tile_pool(name="psA", bufs=1, space="PSUM") as psA_pool,
        ):
            gidx_sb = res_pool.tile([128, PAD // 16], I16)
            dstv_sb = res_pool.tile([128, PAD // 128], BF16)
            scal_sb = res_pool.tile([128, PAD // 128], BF16)
            ciota_sb = res_pool.tile([128, MW], BF16)
            nc.sync.dma_start(out=gidx_sb[:], in_=gidx_d.ap()[:, :])
            nc.sync.dma_start(out=dstv_sb[:], in_=dstv_d.ap()[:, :])
            nc.sync.dma_start(out=scal_sb[:], in_=scal_d.ap()[:, :])
            nc.sync.dma_start(out=ciota_sb[:], in_=ciota_d.ap()[:, :])
            ones_sb = res_pool.tile([1, D], BF16)
            nc.vector.memset(ones_sb[:], 1.0)

            for lay in range(2):
                src_tab = x_rep if lay == 0 else h1rep
                loc_tab = x_loc if lay == 0 else h1b

                ls = nc.enter_named_scope(f"lay_{lay}", False)
                w_sb = wpool.tile([128, (R + 1) * D], BF16, tag="w", bufs=2)
                nc.sync.dma_start(
                    out=w_sb[:].rearrange("d (r e) -> d r e", r=R + 1),
                    in_=w_all.ap()[lay].rearrange("r d e -> d r e"),
                )
                b_sb = wpool.tile([1, D], BF16, tag="b", bufs=2)
                nc.sync.dma_start(out=b_sb[:], in_=b_all.ap()[lay])

                NACC = PAIRS * B
                for bat in range(NBAT):
                    row0 = bat * B * 128
                    if B > 1 or bat % 2 == 0:
                        nrows = min(max(B, 2) * 128, NLP - row0)
                        loct = loct_pool.tile([128, max(B, 2) * 128], BF16,
                                              tag="lt")
                        nc.sync.dma_start_transpose(
                            out=loct[:, :nrows],
                            in_=loc_tab.ap()[row0 : row0 + nrows, :],
                        )
                    # accumulators, each padded to a full PSUM bank so every
                    # concurrently-open accumulation group owns its own 2KB
                    # zero region (PE start=True zeroes the whole region)
                    psA = [
                        psA_pool.tile([128, MW], F32, tag=f"pa{a}",
                                      name=f"psA{a}", bufs=2,
                                      padded_shape=[128, 512])
                        for a in range(NACC)
                    ]
                    for sb in range(NSB):
                        o, n = runs[bat][sb]
                        nch = n // 128
                        msgt = msg_pool.tile([128, nch, D], BF16, tag="msg")
                        for co in range(0, n, MAXC):
                            cn = min(MAXC, n - co)
                            nc.gpsimd.dma_gather(
                                out_ap=msgt[:, co // 128 : (co + cn) // 128, :],
                                in_ap=src_tab.ap()[sb * SBR : (sb + 1) * SBR, :],
                                idxs_ap=gidx_sb[
                                    :, (o + co) // 16 : (o + co + cn) // 16
                                ],
                                num_idxs=cn,
                                num_idxs_reg=cn,
                                elem_size=D,
                                single_packet=SINGLE_PACKET,
                                queue_num=sb % K_QUEUES,
                            )
                        nc.vector.tensor_tensor(
                            out=msgt[:],
                            in0=msgt[:],
                            in1=scal_sb[:, o // 128 : (o + n) // 128, None]
                            .to_broadcast([128, nch, D]),
                            op=mybir.AluOpType.mult,
                        )
                        if K_MDRAM:
                            mt = m_pool.tile([128, nch * MW], FP8, tag="m")
                            nc.sync.dma_start(
                                out=mt[:],
                                in_=m_d.ap()[
                                    :, (o // 128) * MW : ((o + n) // 128) * MW
                                ],
                            )
                        else:
                            mt = m_pool.tile([128, nch * MW], BF16, tag="m")
                            nc.vector.tensor_tensor(
                                out=mt[:].rearrange(
                                    "p (a b) -> p a b", b=MW
                                ),
                                in0=dstv_sb[:, o // 128 : (o + n) // 128, None]
                                .to_broadcast([128, nch, MW]),
                                in1=ciota_sb[:, None, :]
                                .to_broadcast([128, nch, MW]),
                                op=mybir.AluOpType.is_equal,
                            )
                        ci = 0
                        for p in range(PAIRS):
                            for t2 in range(B):
                                gi = ((bat * NSB + sb) * PAIRS + p) * B + t2
                                gch = gsz[gi] // 128
                                for k in range(gch):
                                    nc.tensor.matmul(
                                        out=psA[p * B + t2][:],
                                        lhsT=msgt[:, ci, :],
                                        rhs=mt[:, ci * MW : (ci + 1) * MW],
                                        start=(sb == 0 and k == 0),
                                        stop=(sb == NSB - 1 and k == gch - 1),
                                    )
                                    ci += 1
                        assert ci == nch

                    a_sb = [
                        a_pool.tile([128, MW], BF16, tag=f"a{a}",
                                    name=f"a_sb{a}", bufs=2)
                        for a in range(NACC)
                    ]
                    for a in range(NACC):
                        nc.scalar.activation(
                            out=a_sb[a][:], in_=psA[a][:],
                            func=mybir.ActivationFunctionType.Copy,
                        )

                    for t2 in range(B):
                        # transform reuses a late accumulator's bank (its
                        # group is closed and its data copied to SBUF by now)
                        ps = psA[(PAIRS - 1) * B + t2][:, 0:D]
                        for r in range(R):
                            a0 = (r % K_GRP) * 128
                            nc.tensor.matmul(
                                out=ps,
                                lhsT=a_sb[(r // K_GRP) * B + t2][:, a0 : a0 + 128],
                                rhs=w_sb[:, r * D : (r + 1) * D],
                                start=(r == 0),
                                stop=False,
                            )
                        lc = (bat % 2) * 128 if B == 1 else t2 * 128
                        nc.tensor.matmul(
                            out=ps,
                            lhsT=loct[:, lc : lc + 128],
                            rhs=w_sb[:, R * D : (R + 1) * D],
                            start=False,
                            stop=False,
                        )
                        nc.tensor.matmul(
                            out=ps,
                            lhsT=ones_sb[:1, :],
                            rhs=b_sb[:1, :],
                            start=False,
                            stop=True,
                        )
                        row = row0 + t2 * 128
                        if lay == 0:
                            hs = hpool.tile([128, D], BF16, tag="h0")
                            nc.scalar.activation(
                                out=hs[:], in_=ps,
                                func=mybir.ActivationFunctionType.Relu,
                            )
                            nc.sync.dma_start(
                                out=h1b.ap()[row : row + 128, :], in_=hs[:]
                            )
                        else:
                            nrow = min(128, NL - row)
                            if nrow <= 0:
                                continue
                            hs = hpool.tile([128, D], F32, tag="h1")
                            nc.scalar.activation(
                                out=hs[:], in_=ps,
                                func=mybir.ActivationFunctionType.Relu,
                            )
                            nc.sync.dma_start(
                                out=out_d.ap()[row : row + nrow, :],
                                in_=hs[:nrow, :],
                            )

                nc.leave_named_scope(f"lay_{lay}", ls[0], False)
                if lay == 0:
                    nc.gpsimd.collective_compute(
                        "AllGather",
                        mybir.AluOpType.bypass,
                        replica_groups=[list(range(C))],
                        ins=[h1b.ap()],
                        outs=[h1rep.ap()],
                    )

    nc.compile()
    return nc


# ----------------------------------------------------------------------------
# In-map assembly
# ----------------------------------------------------------------------------
def make_in_maps(prep, W1, root1, b1, W2, root2, b2, cfg):
    C, D, R = cfg["C"], cfg["D"], cfg["R"]
    NL, NT, NLP, NTOT, B, NBAT, PAIRS, NSB, SBR, MW = derive(cfg)
    w_all = np.zeros((2, R + 1, D, D), bf16)
    w_all[0, :R] = np.asarray(W1, np.float32).astype(bf16)
    w_all[0, R] = np.asarray(root1, np.float32).astype(bf16)
    w_all[1, :R] = np.asarray(W2, np.float32).astype(bf16)
    w_all[1, R] = np.asarray(root2, np.float32).astype(bf16)
    b_stack = np.stack([np.asarray(b1, np.float32), np.asarray(b2, np.float32)])
    b_all = b_stack.reshape(2, 1, D).astype(bf16)
    ciota = np.tile(np.arange(MW, dtype=np.float32).astype(bf16), (128, 1))

    in_maps = []
    for c in range(C):
        x_loc = np.ascontiguousarray(prep["x_rep"][c * NLP : (c + 1) * NLP])
        im = {
            "x_rep": prep["x_rep"],
            "x_loc": x_loc,
            "w_all": w_all,
            "b_all": b_all,
            "gidx": prep["gidx"][c],
            "dstv": prep["dstv"][c],
            "scal": prep["scal"][c],
            "ciota": ciota,
        }
        if K_MDRAM:
            im["m_w"] = prep["m_w"][c]
        in_maps.append(im)
    return in_maps


def enable_ntff_hook():
    """Register the axon NTFF profiling hook if the image's antenv lacks it."""
    import sys, types
    try:
        import antenv.axon_hooks  # noqa: F401
        return True
    except ImportError:
        pass
    try:
        from trn_agent_boot.trn_boot import _ntff_profile_via_ctypes
        hook = _ntff_profile_via_ctypes("/opt/axon/libaxon_pjrt.so")
        mod = types.ModuleType("antenv.axon_hooks")
        mod._hook = hook
        mod.set_axon_ntff_profile_hook = lambda h: setattr(mod, "_hook", h)
        mod.get_axon_ntff_profile_hook = lambda: mod._hook
        sys.modules["antenv.axon_hooks"] = mod
        import antenv
        antenv.axon_hooks = mod
        return hook is not None
    except Exception:
        return False


_program_cache = {}


def run(x, edge_index, edge_type, W1, root1, b1, W2, root2, b2,
        cfg=FULL, trace=False):
    prep = host_prep(x, edge_index, edge_type, cfg)
    key = (tuple(sorted(cfg.items())), prep["gsz"], prep["runs"], prep["PAD"],
           K_QUEUES, SINGLE_PACKET, MAXC, K_MDRAM, K_GRP, K_B)
    if key not in _program_cache:
        _program_cache[key] = build_program(
            cfg, prep["gsz"], prep["runs"], prep["PAD"]
        )
    nc = _program_cache[key]
    in_maps = make_in_maps(prep, W1, root1, b1, W2, root2, b2, cfg)
    if trace:
        trace = enable_ntff_hook()
    res = run_bass_kernel_spmd(
        nc, in_maps, core_ids=list(range(cfg["C"])), trace=trace
    )
    blocks = [res.results[c]["out"] for c in range(cfg["C"])]
    full = np.concatenate(blocks, axis=0).astype(np.float32)
    return full, res


def kernel(**inputs):
    out, _ = run(
        inputs["x"], inputs["edge_index"], inputs["edge_type"],
        inputs["W1"], inputs["root1"], inputs["b1"],
        inputs["W2"], inputs["root2"], inputs["b2"],
    )
    return out


# revision 24
# speedup vs baseline: 2.3492x; 1.0415x over previous
"""RGCN (2-layer, mean-aggregation) Bass kernel for one TRN2 chip (8 NeuronCores).

Strategy (dst-sharded, matmul-based aggregation — no DRAM scatter):
  - Nodes block-partitioned across 8 cores (12500/core, padded to 12544).
    Edges live on their dst-owner core; x is replicated (bf16, padded layout
    [C*NLP, D]) in every core's HBM.
  - Edges are sorted by (dst-tile-batch, src-quarter, rel-pair, tile) and
    padded per group to 128. Per (batch, src-quarter) run: dma_gather the
    messages x[src] ([128e, D] natural layout), scale by inv_deg (DVE), build
    a one-hot matrix M[e, (rel%2)*128 + dst%128] with a DVE is_equal against
    a resident iota row, then aggregate on the tensor engine:
        A_pairT[d, code] += msg^T @ M   (PSUM, f32 accumulation)
    A_pairT is directly the lhsT for the transform matmuls:
        out[dst, :] = relu(sum_r A_rT.T @ W_r + x_locT.T @ Wroot + b)
  - The dst one-hot lives in rhs (free dim 256 = rel-parity x dst128), PSUM
    holds 4 rel-pair accumulators per 2-tile batch (4 banks).
  - Gathers use int16 indices over 4 equal row-quarters of the replicated
    table (25088 rows < 32768), one SWDGE queue per quarter.
  - Between layers one small AllGather (bf16, 3.2MB/rank) replicates the new
    features; layer 1's replication is free (host pre-stages x_rep).
"""

import os
import numpy as np
import ml_dtypes

import concourse.tile as tile
from concourse import bass, bacc, mybir
from concourse.bass_utils import run_bass_kernel_spmd

BF16 = mybir.dt.bfloat16
F32 = mybir.dt.float32
I16 = mybir.dt.int16
bf16 = ml_dtypes.bfloat16

K_QUEUES = int(os.environ.get("K_QUEUES", "4"))
SINGLE_PACKET = os.environ.get("K_SP", "0") == "1"
MAXC = int(os.environ.get("K_MAXC", "1024" if SINGLE_PACKET else "1920"))
# M source: 0 = build one-hot on DVE (is_equal), 1 = DMA precomputed fp8
K_MDRAM = int(os.environ.get("K_MDRAM", "1"))
K_GRP = int(os.environ.get("K_GRP", "4"))   # relations per accumulator group
K_B = int(os.environ.get("K_B", "2"))       # dst tiles per batch
FP8 = mybir.dt.float8e4
fp8 = ml_dtypes.float8_e4m3fn

# ----------------------------------------------------------------------------
# Problem constants
# ----------------------------------------------------------------------------
FULL = dict(N=100000, E=1000000, D=128, R=8, C=8)


def derive(cfg):
    N, D, R, C = cfg["N"], cfg["D"], cfg["R"], cfg["C"]
    NL = N // C                      # owned nodes per core
    NT = (NL + 127) // 128           # dst tiles per core
    NLP = NT * 128                   # padded rows per block
    NTOT = C * NLP                   # replicated-table rows
    B = K_B                          # dst tiles per batch
    NBAT = NT // B                   # batches per core
    PAIRS = R // K_GRP               # accumulator groups per (tile)
    NSB = 4                          # src table quarters
    SBR = NTOT // NSB                # rows per quarter (must be < 32768)
    MW = K_GRP * 128                 # M width: (rel%GRP)*128 + dst%128
    # PSUM: PAIRS*B accumulators, each padded to a full 2KB bank, x2 bufs
    assert PAIRS * B * 2 <= 8 and MW <= 512
    return NL, NT, NLP, NTOT, B, NBAT, PAIRS, NSB, SBR, MW


# ----------------------------------------------------------------------------
# Host-side preprocessing
# ----------------------------------------------------------------------------
def host_prep(x, edge_index, edge_type, cfg):
    N, E, D, R, C = cfg["N"], cfg["E"], cfg["D"], cfg["R"], cfg["C"]
    NL, NT, NLP, NTOT, B, NBAT, PAIRS, NSB, SBR, MW = derive(cfg)
    assert SBR < 32768 and NT % B == 0

    src = np.asarray(edge_index[0], dtype=np.int64)
    dst = np.asarray(edge_index[1], dtype=np.int64)
    et = np.asarray(edge_type, dtype=np.int64)

    # mean-normalization per (relation, dst), computed on host (graph-only)
    deg = np.zeros((R, N), np.float32)
    np.add.at(deg, (et, dst), 1.0)
    inv = np.where(deg > 0, 1.0 / np.maximum(deg, 1.0), 0.0).astype(np.float32)
    scale_e = inv[et, dst]

    core = dst // NL
    dl = dst % NL
    tl = dl // 128
    bat = tl // B
    t2 = tl % B
    pair = et // K_GRP
    code = (et % K_GRP) * 128 + (dl % 128)
    srcp = (src // NL) * NLP + (src % NL)       # padded replicated-table row
    sb = srcp // SBR
    sidx = (srcp % SBR).astype(np.int64)

    NG = NBAT * NSB * PAIRS * B
    g = ((bat * NSB + sb) * PAIRS + pair) * B + t2

    counts = np.zeros((C, NG), np.int64)
    np.add.at(counts, (core, g), 1)
    gsz = np.maximum(((counts.max(axis=0) + 127) // 128) * 128, 128)  # [NG]
    offs = np.zeros(NG + 1, np.int64)
    np.cumsum(gsz, out=offs[1:])
    PAD = int(offs[-1])

    # place edges: stable sort by (core, g), rank within each (core, g) run
    key = core * NG + g
    order = np.argsort(key, kind="stable")
    key_o = key[order]
    new_run = np.ones(E, bool)
    new_run[1:] = key_o[1:] != key_o[:-1]
    run_starts = np.flatnonzero(new_run)
    run_id = np.cumsum(new_run) - 1
    rank = np.arange(E) - run_starts[run_id]
    pos = offs[g[order]] + rank

    gidx_a = np.zeros((C, PAD), np.int16)
    code_i = np.full((C, PAD), 10000, np.int32)   # pad sentinel, out of range
    scal_a = np.zeros((C, PAD), bf16)
    co = core[order]
    gidx_a[co, pos] = sidx[order].astype(np.int16)
    code_i[co, pos] = code[order].astype(np.int32)
    scal_a[co, pos] = scale_e[order].astype(bf16)
    # bf16 codes only feed the K_MDRAM=0 is_equal path; bf16 integers are
    # exact to 256 so that path requires MW <= 256
    assert K_MDRAM or MW <= 256
    code_a = np.minimum(code_i, 300).astype(bf16)

    # wrapped layouts: idx i at [i%16, i//16] (x8 down); code/scale at
    # [i%128, i//128]
    gidx_w = np.tile(
        gidx_a.reshape(C, PAD // 16, 16).transpose(0, 2, 1), (1, 8, 1)
    )
    dstv_w = np.ascontiguousarray(
        code_a.reshape(C, PAD // 128, 128).transpose(0, 2, 1)
    )
    scal_w = np.ascontiguousarray(
        scal_a.reshape(C, PAD // 128, 128).transpose(0, 2, 1)
    )

    # precomputed one-hot M (fp8), wrapped [C, 128, (PAD//128)*MW]
    m_w = None
    if K_MDRAM:
        m_full = (
            code_i.reshape(C, PAD // 128, 128)[..., None]
            == np.arange(MW, dtype=np.int32)
        )
        m_w = np.ascontiguousarray(
            m_full.transpose(0, 2, 1, 3).reshape(C, 128, (PAD // 128) * MW)
        ).astype(fp8)

    # replicated, block-padded x (bf16): [NTOT, D]
    x = np.asarray(x, np.float32)
    x_rep = np.zeros((NTOT, D), bf16)
    for c in range(C):
        x_rep[c * NLP : c * NLP + NL] = x[c * NL : (c + 1) * NL].astype(bf16)

    # run table: per (bat, sb): (offset, size); groups within are (pair, t2)
    runs = []
    for b_ in range(NBAT):
        row = []
        for s_ in range(NSB):
            g0 = ((b_ * NSB + s_) * PAIRS) * B
            o = int(offs[g0])
            n = int(offs[g0 + PAIRS * B] - offs[g0])
            row.append((o, n))
        runs.append(row)

    return dict(
        gsz=tuple(int(v) for v in gsz),
        runs=tuple(tuple(r) for r in runs),
        PAD=PAD,
        gidx=np.ascontiguousarray(gidx_w),
        dstv=dstv_w,
        scal=scal_w,
        m_w=m_w,
        code_i=code_i,
        x_rep=x_rep,
    )


# ----------------------------------------------------------------------------
# Device program
# ----------------------------------------------------------------------------
def build_program(cfg, gsz, runs, PAD):
    N, E, D, R, C = cfg["N"], cfg["E"], cfg["D"], cfg["R"], cfg["C"]
    NL, NT, NLP, NTOT, B, NBAT, PAIRS, NSB, SBR, MW = derive(cfg)

    nc = bacc.Bacc(
        "TRN2", target_bir_lowering=False, debug=False,
        enable_asserts=False, num_devices=C, num_swdge_queues=K_QUEUES,
    )

    x_rep = nc.dram_tensor("x_rep", [NTOT, D], BF16, kind="ExternalInput")
    x_loc = nc.dram_tensor("x_loc", [NLP, D], BF16, kind="ExternalInput")
    w_all = nc.dram_tensor("w_all", [2, R + 1, D, D], BF16, kind="ExternalInput")
    b_all = nc.dram_tensor("b_all", [2, 1, D], BF16, kind="ExternalInput")
    gidx_d = nc.dram_tensor("gidx", [128, PAD // 16], I16, kind="ExternalInput")
    dstv_d = nc.dram_tensor("dstv", [128, PAD // 128], BF16, kind="ExternalInput")
    scal_d = nc.dram_tensor("scal", [128, PAD // 128], BF16, kind="ExternalInput")
    ciota_d = nc.dram_tensor("ciota", [128, MW], BF16, kind="ExternalInput")
    if K_MDRAM:
        m_d = nc.dram_tensor(
            "m_w", [128, (PAD // 128) * MW], FP8, kind="ExternalInput"
        )
    out_d = nc.dram_tensor("out", [NL, D], F32, kind="ExternalOutput")
    h1b = nc.dram_tensor("h1b", [NLP, D], BF16, kind="Internal")
    h1rep = nc.dram_tensor(
        "h1rep", [NTOT, D], BF16, kind="Internal", addr_space="Shared"
    )

    with tile.TileContext(nc) as tc:
        with (
            tc.tile_pool(name="resident", bufs=1) as res_pool,
            tc.tile_pool(name="msg", bufs=8) as msg_pool,
            tc.tile_pool(name="mm", bufs=8) as m_pool,
            tc.tile_pool(name="asb", bufs=2) as a_pool,
            tc.tile_pool(name="loct", bufs=2) as loct_pool,
            tc.tile_pool(name="wpool", bufs=1) as wpool,
            tc.tile_pool(name="hout", bufs=4) as hpool,
            tc.tile_pool(name="psA", bufs=1, space="PSUM") as psA_pool,
        ):
            gidx_sb = res_pool.tile([128, PAD // 16], I16)
            dstv_sb = res_pool.tile([128, PAD // 128], BF16)
            scal_sb = res_pool.tile([128, PAD // 128], BF16)
            ciota_sb = res_pool.tile([128, MW], BF16)
            nc.sync.dma_start(out=gidx_sb[:], in_=gidx_d.ap()[:, :])
            nc.sync.dma_start(out=dstv_sb[:], in_=dstv_d.ap()[:, :])
            nc.sync.dma_start(out=scal_sb[:], in_=scal_d.ap()[:, :])
            nc.sync.dma_start(out=ciota_sb[:], in_=ciota_d.ap()[:, :])
            ones_sb = res_pool.tile([1, D], BF16)
            nc.vector.memset(ones_sb[:], 1.0)

            for lay in range(2):
                src_tab = x_rep if lay == 0 else h1rep
                loc_tab = x_loc if lay == 0 else h1b

                ls = nc.enter_named_scope(f"lay_{lay}", False)
                w_sb = wpool.tile([128, (R + 1) * D], BF16, tag="w", bufs=2)
                nc.sync.dma_start(
                    out=w_sb[:].rearrange("d (r e) -> d r e", r=R + 1),
                    in_=w_all.ap()[lay].rearrange("r d e -> d r e"),
                )
                b_sb = wpool.tile([1, D], BF16, tag="b", bufs=2)
                nc.sync.dma_start(out=b_sb[:], in_=b_all.ap()[lay])

                NACC = PAIRS * B
                for bat in range(NBAT):
                    row0 = bat * B * 128
                    if B > 1 or bat % 2 == 0:
                        nrows = min(max(B, 2) * 128, NLP - row0)
                        loct = loct_pool.tile([128, max(B, 2) * 128], BF16,
                                              tag="lt")
                        nc.sync.dma_start_transpose(
                            out=loct[:, :nrows],
                            in_=loc_tab.ap()[row0 : row0 + nrows, :],
                        )
                    # accumulators, each padded to a full PSUM bank so every
                    # concurrently-open accumulation group owns its own 2KB
                    # zero region (PE start=True zeroes the whole region)
                    psA = [
                        psA_pool.tile([128, MW], F32, tag=f"pa{a}",
                                      name=f"psA{a}", bufs=2,
                                      padded_shape=[128, 512])
                        for a in range(NACC)
                    ]
                    for sb in range(NSB):
                        o, n = runs[bat][sb]
                        nch = n // 128
                        msgt = msg_pool.tile([128, nch, D], BF16, tag="msg")
                        for co in range(0, n, MAXC):
                            cn = min(MAXC, n - co)
                            nc.gpsimd.dma_gather(
                                out_ap=msgt[:, co // 128 : (co + cn) // 128, :],
                                in_ap=src_tab.ap()[sb * SBR : (sb + 1) * SBR, :],
                                idxs_ap=gidx_sb[
                                    :, (o + co) // 16 : (o + co + cn) // 16
                                ],
                                num_idxs=cn,
                                num_idxs_reg=cn,
                                elem_size=D,
                                single_packet=SINGLE_PACKET,
                                queue_num=sb % K_QUEUES,
                            )
                        nc.vector.tensor_tensor(
                            out=msgt[:],
                            in0=msgt[:],
                            in1=scal_sb[:, o // 128 : (o + n) // 128, None]
                            .to_broadcast([128, nch, D]),
                            op=mybir.AluOpType.mult,
                        )
                        if K_MDRAM:
                            mt = m_pool.tile([128, nch * MW], FP8, tag="m")
                            nc.sync.dma_start(
                                out=mt[:],
                                in_=m_d.ap()[
                                    :, (o // 128) * MW : ((o + n) // 128) * MW
                                ],
                            )
                        else:
                            mt = m_pool.tile([128, nch * MW], BF16, tag="m")
                            nc.vector.tensor_tensor(
                                out=mt[:].rearrange(
                                    "p (a b) -> p a b", b=MW
                                ),
                                in0=dstv_sb[:, o // 128 : (o + n) // 128, None]
                                .to_broadcast([128, nch, MW]),
                                in1=ciota_sb[:, None, :]
                                .to_broadcast([128, nch, MW]),
                                op=mybir.AluOpType.is_equal,
                            )
                        ci = 0
                        for p in range(PAIRS):
                            for t2 in range(B):
                                gi = ((bat * NSB + sb) * PAIRS + p) * B + t2
                                gch = gsz[gi] // 128
                                for k in range(gch):
                                    nc.tensor.matmul(
                                        out=psA[p * B + t2][:],
                                        lhsT=msgt[:, ci, :],
                                        rhs=mt[:, ci * MW : (ci + 1) * MW],
                                        start=(sb == 0 and k == 0),
                                        stop=(sb == NSB - 1 and k == gch - 1),
                                    )
                                    ci += 1
                        assert ci == nch

                    a_sb = [
                        a_pool.tile([128, MW], BF16, tag=f"a{a}",
                                    name=f"a_sb{a}", bufs=2)
                        for a in range(NACC)
                    ]
                    for a in range(NACC):
                        nc.scalar.activation(
                            out=a_sb[a][:], in_=psA[a][:],
                            func=mybir.ActivationFunctionType.Copy,
                        )

                    for t2 in range(B):
                        # transform reuses a late accumulator's bank (its
                        # group is closed and its data copied to SBUF by now)
                        ps = psA[(PAIRS - 1) * B + t2][:, 0:D]
                        for r in range(R):
                            a0 = (r % K_GRP) * 128
                            nc.tensor.matmul(
                                out=ps,
                                lhsT=a_sb[(r // K_GRP) * B + t2][:, a0 : a0 + 128],
                                rhs=w_sb[:, r * D : (r + 1) * D],
                                start=(r == 0),
                                stop=False,
                            )
                        lc = (bat % 2) * 128 if B == 1 else t2 * 128
                        nc.tensor.matmul(
                            out=ps,
                            lhsT=loct[:, lc : lc + 128],
                            rhs=w_sb[:, R * D : (R + 1) * D],
                            start=False,
                            stop=False,
                        )
                        nc.tensor.matmul(
                            out=ps,
                            lhsT=ones_sb[:1, :],
                            rhs=b_sb[:1, :],
                            start=False,
                            stop=True,
                        )
                        row = row0 + t2 * 128
                        if lay == 0:
                            hs = hpool.tile([128, D], BF16, tag="h0")
                            nc.scalar.activation(
                                out=hs[:], in_=ps,
                                func=mybir.ActivationFunctionType.Relu,
                            )
                            nc.sync.dma_start(
                                out=h1b.ap()[row : row + 128, :], in_=hs[:]
                            )
                        else:
                            nrow = min(128, NL - row)
                            if nrow <= 0:
                                continue
                            hs = hpool.tile([128, D], F32, tag="h1")
                            nc.scalar.activation(
                                out=hs[:], in_=ps,
                                func=mybir.ActivationFunctionType.Relu,
                            )
                            nc.sync.dma_start(
                                out=out_d.ap()[row : row + nrow, :],
                                in_=hs[:nrow, :],
                            )

                nc.leave_named_scope(f"lay_{lay}", ls[0], False)
                if lay == 0:
                    nc.gpsimd.collective_compute(
                        "AllGather",
                        mybir.AluOpType.bypass,
                        replica_groups=[list(range(C))],
                        ins=[h1b.ap()],
                        outs=[h1rep.ap()],
                    )

    nc.compile()
    return nc


# ----------------------------------------------------------------------------
# In-map assembly
# ----------------------------------------------------------------------------
def make_in_maps(prep, W1, root1, b1, W2, root2, b2, cfg):
    C, D, R = cfg["C"], cfg["D"], cfg["R"]
    NL, NT, NLP, NTOT, B, NBAT, PAIRS, NSB, SBR, MW = derive(cfg)
    w_all = np.zeros((2, R + 1, D, D), bf16)
    w_all[0, :R] = np.asarray(W1, np.float32).astype(bf16)
    w_all[0, R] = np.asarray(root1, np.float32).astype(bf16)
    w_all[1, :R] = np.asarray(W2, np.float32).astype(bf16)
    w_all[1, R] = np.asarray(root2, np.float32).astype(bf16)
    b_stack = np.stack([np.asarray(b1, np.float32), np.asarray(b2, np.float32)])
    b_all = b_stack.reshape(2, 1, D).astype(bf16)
    ciota = np.tile(np.arange(MW, dtype=np.float32).astype(bf16), (128, 1))

    in_maps = []
    for c in range(C):
        x_loc = np.ascontiguousarray(prep["x_rep"][c * NLP : (c + 1) * NLP])
        im = {
            "x_rep": prep["x_rep"],
            "x_loc": x_loc,
            "w_all": w_all,
            "b_all": b_all,
            "gidx": prep["gidx"][c],
            "dstv": prep["dstv"][c],
            "scal": prep["scal"][c],
            "ciota": ciota,
        }
        if K_MDRAM:
            im["m_w"] = prep["m_w"][c]
        in_maps.append(im)
    return in_maps


def enable_ntff_hook():
    """Register the axon NTFF profiling hook if the image's antenv lacks it."""
    import sys, types
    try:
        import antenv.axon_hooks  # noqa: F401
        return True
    except ImportError:
        pass
    try:
        from trn_agent_boot.trn_boot import _ntff_profile_via_ctypes
        hook = _ntff_profile_via_ctypes("/opt/axon/libaxon_pjrt.so")
        mod = types.ModuleType("antenv.axon_hooks")
        mod._hook = hook
        mod.set_axon_ntff_profile_hook = lambda h: setattr(mod, "_hook", h)
        mod.get_axon_ntff_profile_hook = lambda: mod._hook
        sys.modules["antenv.axon_hooks"] = mod
        import antenv
        antenv.axon_hooks = mod
        return hook is not None
    except Exception:
        return False


_program_cache = {}


def run(x, edge_index, edge_type, W1, root1, b1, W2, root2, b2,
        cfg=FULL, trace=False):
    prep = host_prep(x, edge_index, edge_type, cfg)
    key = (tuple(sorted(cfg.items())), prep["gsz"], prep["runs"], prep["PAD"],
           K_QUEUES, SINGLE_PACKET, MAXC, K_MDRAM, K_GRP, K_B)
    if key not in _program_cache:
        _program_cache[key] = build_program(
            cfg, prep["gsz"], prep["runs"], prep["PAD"]
        )
    nc = _program_cache[key]
    in_maps = make_in_maps(prep, W1, root1, b1, W2, root2, b2, cfg)
    if trace:
        trace = enable_ntff_hook()
    res = run_bass_kernel_spmd(
        nc, in_maps, core_ids=list(range(cfg["C"])), trace=trace
    )
    blocks = [res.results[c]["out"] for c in range(cfg["C"])]
    full = np.concatenate(blocks, axis=0).astype(np.float32)
    return full, res


def kernel(**inputs):
    out, _ = run(
        inputs["x"], inputs["edge_index"], inputs["edge_type"],
        inputs["W1"], inputs["root1"], inputs["b1"],
        inputs["W2"], inputs["root2"], inputs["b2"],
    )
    return out


# revision 26
# speedup vs baseline: 2.4271x; 1.0332x over previous
"""RGCN (2-layer, mean-aggregation) Bass kernel for one TRN2 chip (8 NeuronCores).

Strategy (dst-sharded, matmul-based aggregation — no DRAM scatter):
  - Nodes block-partitioned across 8 cores (12500/core, padded to 12544).
    Edges live on their dst-owner core; x is replicated (bf16, padded layout
    [C*NLP, D]) in every core's HBM.
  - Edges are sorted by (dst tile, src-quarter, rel-group) and padded per
    group to a multiple of 128. Per (tile, src-quarter) run: dma_gather the
    messages x[src] ([128e, D] natural layout), scale by inv_deg (DVE), build
    a one-hot matrix M[e, (rel%GRP)*128 + dst%128] (DVE is_equal against a
    resident iota row, or DMA a host-precomputed fp8 M when K_MDRAM=1), then
    aggregate on the tensor engine:
        A_grpT[d, code] += msg^T @ M   (PSUM, f32 accumulation)
    A_grpT is directly the lhsT for the transform matmuls:
        out[dst, :] = relu(sum_r A_rT.T @ W_r + x_locT.T @ Wroot + b)
  - PSUM: each accumulator is padded to a full 2KB bank (PE start=True zeroes
    the whole 2KB region, so concurrently-open groups must not share a bank);
    accumulators are double-buffered and the transform output aliases into a
    closed accumulator bank.
  - Gathers use int16 indices over 4 equal row-quarters of the replicated
    table (25088 rows < 32768), one SWDGE queue per quarter; deep tile-pool
    buffering (bufs=8) keeps a full tile of 4 gather runs in flight.
  - Between layers one small AllGather (bf16, 3.2MB/rank) replicates the new
    features; layer 1's replication is free (host pre-stages x_rep).
"""

import os
import numpy as np
import ml_dtypes

import concourse.tile as tile
from concourse import bass, bacc, mybir
from concourse.bass_utils import run_bass_kernel_spmd

BF16 = mybir.dt.bfloat16
F32 = mybir.dt.float32
I16 = mybir.dt.int16
bf16 = ml_dtypes.bfloat16

K_QUEUES = int(os.environ.get("K_QUEUES", "4"))
SINGLE_PACKET = os.environ.get("K_SP", "0") == "1"
MAXC = int(os.environ.get("K_MAXC", "1024" if SINGLE_PACKET else "1920"))
# M source: 0 = build one-hot on DVE (is_equal), 1 = DMA precomputed fp8
K_MDRAM = int(os.environ.get("K_MDRAM", "0"))
K_GRP = int(os.environ.get("K_GRP", "2"))   # relations per accumulator group
K_B = int(os.environ.get("K_B", "1"))       # dst tiles per batch
FP8 = mybir.dt.float8e4
fp8 = ml_dtypes.float8_e4m3fn

# ----------------------------------------------------------------------------
# Problem constants
# ----------------------------------------------------------------------------
FULL = dict(N=100000, E=1000000, D=128, R=8, C=8)


def derive(cfg):
    N, D, R, C = cfg["N"], cfg["D"], cfg["R"], cfg["C"]
    NL = N // C                      # owned nodes per core
    NT = (NL + 127) // 128           # dst tiles per core
    NLP = NT * 128                   # padded rows per block
    NTOT = C * NLP                   # replicated-table rows
    B = K_B                          # dst tiles per batch
    NBAT = NT // B                   # batches per core
    PAIRS = R // K_GRP               # accumulator groups per (tile)
    NSB = 4                          # src table quarters
    SBR = NTOT // NSB                # rows per quarter (must be < 32768)
    MW = K_GRP * 128                 # M width: (rel%GRP)*128 + dst%128
    # PSUM: PAIRS*B accumulators, each padded to a full 2KB bank, x2 bufs
    assert PAIRS * B * 2 <= 8 and MW <= 512
    return NL, NT, NLP, NTOT, B, NBAT, PAIRS, NSB, SBR, MW


# ----------------------------------------------------------------------------
# Host-side preprocessing
# ----------------------------------------------------------------------------
def host_prep(x, edge_index, edge_type, cfg):
    N, E, D, R, C = cfg["N"], cfg["E"], cfg["D"], cfg["R"], cfg["C"]
    NL, NT, NLP, NTOT, B, NBAT, PAIRS, NSB, SBR, MW = derive(cfg)
    assert SBR < 32768 and NT % B == 0

    src = np.asarray(edge_index[0], dtype=np.int64)
    dst = np.asarray(edge_index[1], dtype=np.int64)
    et = np.asarray(edge_type, dtype=np.int64)

    # mean-normalization per (relation, dst), computed on host (graph-only)
    deg = np.zeros((R, N), np.float32)
    np.add.at(deg, (et, dst), 1.0)
    inv = np.where(deg > 0, 1.0 / np.maximum(deg, 1.0), 0.0).astype(np.float32)
    scale_e = inv[et, dst]

    core = dst // NL
    dl = dst % NL
    tl = dl // 128
    bat = tl // B
    t2 = tl % B
    pair = et // K_GRP
    code = (et % K_GRP) * 128 + (dl % 128)
    srcp = (src // NL) * NLP + (src % NL)       # padded replicated-table row
    sb = srcp // SBR
    sidx = (srcp % SBR).astype(np.int64)

    NG = NBAT * NSB * PAIRS * B
    g = ((bat * NSB + sb) * PAIRS + pair) * B + t2

    counts = np.zeros((C, NG), np.int64)
    np.add.at(counts, (core, g), 1)
    gsz = np.maximum(((counts.max(axis=0) + 127) // 128) * 128, 128)  # [NG]
    offs = np.zeros(NG + 1, np.int64)
    np.cumsum(gsz, out=offs[1:])
    PAD = int(offs[-1])

    # place edges: stable sort by (core, g), rank within each (core, g) run
    key = core * NG + g
    order = np.argsort(key, kind="stable")
    key_o = key[order]
    new_run = np.ones(E, bool)
    new_run[1:] = key_o[1:] != key_o[:-1]
    run_starts = np.flatnonzero(new_run)
    run_id = np.cumsum(new_run) - 1
    rank = np.arange(E) - run_starts[run_id]
    pos = offs[g[order]] + rank

    gidx_a = np.zeros((C, PAD), np.int16)
    code_i = np.full((C, PAD), 10000, np.int32)   # pad sentinel, out of range
    scal_a = np.zeros((C, PAD), bf16)
    co = core[order]
    gidx_a[co, pos] = sidx[order].astype(np.int16)
    code_i[co, pos] = code[order].astype(np.int32)
    scal_a[co, pos] = scale_e[order].astype(bf16)
    # bf16 codes only feed the K_MDRAM=0 is_equal path; bf16 integers are
    # exact to 256 so that path requires MW <= 256
    assert K_MDRAM or MW <= 256
    code_a = np.minimum(code_i, 300).astype(bf16)

    # wrapped layouts: idx i at [i%16, i//16] (x8 down); code/scale at
    # [i%128, i//128]
    gidx_w = np.tile(
        gidx_a.reshape(C, PAD // 16, 16).transpose(0, 2, 1), (1, 8, 1)
    )
    dstv_w = np.ascontiguousarray(
        code_a.reshape(C, PAD // 128, 128).transpose(0, 2, 1)
    )
    scal_w = np.ascontiguousarray(
        scal_a.reshape(C, PAD // 128, 128).transpose(0, 2, 1)
    )

    # precomputed one-hot M (fp8), wrapped [C, 128, (PAD//128)*MW]
    m_w = None
    if K_MDRAM:
        m_full = (
            code_i.reshape(C, PAD // 128, 128)[..., None]
            == np.arange(MW, dtype=np.int32)
        )
        m_w = np.ascontiguousarray(
            m_full.transpose(0, 2, 1, 3).reshape(C, 128, (PAD // 128) * MW)
        ).astype(fp8)

    # replicated, block-padded x (bf16): [NTOT, D]
    x = np.asarray(x, np.float32)
    x_rep = np.zeros((NTOT, D), bf16)
    for c in range(C):
        x_rep[c * NLP : c * NLP + NL] = x[c * NL : (c + 1) * NL].astype(bf16)

    # run table: per (bat, sb): (offset, size); groups within are (pair, t2)
    runs = []
    for b_ in range(NBAT):
        row = []
        for s_ in range(NSB):
            g0 = ((b_ * NSB + s_) * PAIRS) * B
            o = int(offs[g0])
            n = int(offs[g0 + PAIRS * B] - offs[g0])
            row.append((o, n))
        runs.append(row)

    return dict(
        gsz=tuple(int(v) for v in gsz),
        runs=tuple(tuple(r) for r in runs),
        PAD=PAD,
        gidx=np.ascontiguousarray(gidx_w),
        dstv=dstv_w,
        scal=scal_w,
        m_w=m_w,
        code_i=code_i,
        x_rep=x_rep,
    )


# ----------------------------------------------------------------------------
# Device program
# ----------------------------------------------------------------------------
def build_program(cfg, gsz, runs, PAD):
    N, E, D, R, C = cfg["N"], cfg["E"], cfg["D"], cfg["R"], cfg["C"]
    NL, NT, NLP, NTOT, B, NBAT, PAIRS, NSB, SBR, MW = derive(cfg)

    nc = bacc.Bacc(
        "TRN2", target_bir_lowering=False, debug=False,
        enable_asserts=False, num_devices=C, num_swdge_queues=K_QUEUES,
    )

    x_rep = nc.dram_tensor("x_rep", [NTOT, D], BF16, kind="ExternalInput")
    x_loc = nc.dram_tensor("x_loc", [NLP, D], BF16, kind="ExternalInput")
    w_all = nc.dram_tensor("w_all", [2, R + 1, D, D], BF16, kind="ExternalInput")
    b_all = nc.dram_tensor("b_all", [2, 1, D], BF16, kind="ExternalInput")
    gidx_d = nc.dram_tensor("gidx", [128, PAD // 16], I16, kind="ExternalInput")
    dstv_d = nc.dram_tensor("dstv", [128, PAD // 128], BF16, kind="ExternalInput")
    scal_d = nc.dram_tensor("scal", [128, PAD // 128], BF16, kind="ExternalInput")
    ciota_d = nc.dram_tensor("ciota", [128, MW], BF16, kind="ExternalInput")
    if K_MDRAM:
        m_d = nc.dram_tensor(
            "m_w", [128, (PAD // 128) * MW], FP8, kind="ExternalInput"
        )
    out_d = nc.dram_tensor("out", [NL, D], F32, kind="ExternalOutput")
    h1b = nc.dram_tensor("h1b", [NLP, D], BF16, kind="Internal")
    h1rep = nc.dram_tensor(
        "h1rep", [NTOT, D], BF16, kind="Internal", addr_space="Shared"
    )

    with tile.TileContext(nc) as tc:
        with (
            tc.tile_pool(name="resident", bufs=1) as res_pool,
            tc.tile_pool(name="msg", bufs=8) as msg_pool,
            tc.tile_pool(name="mm", bufs=8) as m_pool,
            tc.tile_pool(name="asb", bufs=2) as a_pool,
            tc.tile_pool(name="loct", bufs=2) as loct_pool,
            tc.tile_pool(name="wpool", bufs=1) as wpool,
            tc.tile_pool(name="hout", bufs=4) as hpool,
            tc.tile_pool(name="psA", bufs=1, space="PSUM") as psA_pool,
        ):
            gidx_sb = res_pool.tile([128, PAD // 16], I16)
            dstv_sb = res_pool.tile([128, PAD // 128], BF16)
            scal_sb = res_pool.tile([128, PAD // 128], BF16)
            ciota_sb = res_pool.tile([128, MW], BF16)
            nc.sync.dma_start(out=gidx_sb[:], in_=gidx_d.ap()[:, :])
            nc.sync.dma_start(out=dstv_sb[:], in_=dstv_d.ap()[:, :])
            nc.sync.dma_start(out=scal_sb[:], in_=scal_d.ap()[:, :])
            nc.sync.dma_start(out=ciota_sb[:], in_=ciota_d.ap()[:, :])
            ones_sb = res_pool.tile([1, D], BF16)
            nc.vector.memset(ones_sb[:], 1.0)

            for lay in range(2):
                src_tab = x_rep if lay == 0 else h1rep
                loc_tab = x_loc if lay == 0 else h1b

                ls = nc.enter_named_scope(f"lay_{lay}", False)
                w_sb = wpool.tile([128, (R + 1) * D], BF16, tag="w", bufs=2)
                nc.sync.dma_start(
                    out=w_sb[:].rearrange("d (r e) -> d r e", r=R + 1),
                    in_=w_all.ap()[lay].rearrange("r d e -> d r e"),
                )
                b_sb = wpool.tile([1, D], BF16, tag="b", bufs=2)
                nc.sync.dma_start(out=b_sb[:], in_=b_all.ap()[lay])

                NACC = PAIRS * B
                for bat in range(NBAT):
                    row0 = bat * B * 128
                    if B > 1 or bat % 2 == 0:
                        nrows = min(max(B, 2) * 128, NLP - row0)
                        loct = loct_pool.tile([128, max(B, 2) * 128], BF16,
                                              tag="lt")
                        nc.sync.dma_start_transpose(
                            out=loct[:, :nrows],
                            in_=loc_tab.ap()[row0 : row0 + nrows, :],
                        )
                    # accumulators, each padded to a full PSUM bank so every
                    # concurrently-open accumulation group owns its own 2KB
                    # zero region (PE start=True zeroes the whole region)
                    psA = [
                        psA_pool.tile([128, MW], F32, tag=f"pa{a}",
                                      name=f"psA{a}", bufs=2,
                                      padded_shape=[128, 512])
                        for a in range(NACC)
                    ]
                    for sb in range(NSB):
                        o, n = runs[bat][sb]
                        nch = n // 128
                        msgt = msg_pool.tile([128, nch, D], BF16, tag="msg")
                        for co in range(0, n, MAXC):
                            cn = min(MAXC, n - co)
                            nc.gpsimd.dma_gather(
                                out_ap=msgt[:, co // 128 : (co + cn) // 128, :],
                                in_ap=src_tab.ap()[sb * SBR : (sb + 1) * SBR, :],
                                idxs_ap=gidx_sb[
                                    :, (o + co) // 16 : (o + co + cn) // 16
                                ],
                                num_idxs=cn,
                                num_idxs_reg=cn,
                                elem_size=D,
                                single_packet=SINGLE_PACKET,
                                queue_num=sb % K_QUEUES,
                            )
                        nc.vector.tensor_tensor(
                            out=msgt[:],
                            in0=msgt[:],
                            in1=scal_sb[:, o // 128 : (o + n) // 128, None]
                            .to_broadcast([128, nch, D]),
                            op=mybir.AluOpType.mult,
                        )
                        if K_MDRAM:
                            mt = m_pool.tile([128, nch * MW], FP8, tag="m")
                            nc.sync.dma_start(
                                out=mt[:],
                                in_=m_d.ap()[
                                    :, (o // 128) * MW : ((o + n) // 128) * MW
                                ],
                            )
                        else:
                            mt = m_pool.tile([128, nch * MW], BF16, tag="m")
                            nc.vector.tensor_tensor(
                                out=mt[:].rearrange(
                                    "p (a b) -> p a b", b=MW
                                ),
                                in0=dstv_sb[:, o // 128 : (o + n) // 128, None]
                                .to_broadcast([128, nch, MW]),
                                in1=ciota_sb[:, None, :]
                                .to_broadcast([128, nch, MW]),
                                op=mybir.AluOpType.is_equal,
                            )
                        ci = 0
                        for p in range(PAIRS):
                            for t2 in range(B):
                                gi = ((bat * NSB + sb) * PAIRS + p) * B + t2
                                gch = gsz[gi] // 128
                                for k in range(gch):
                                    nc.tensor.matmul(
                                        out=psA[p * B + t2][:],
                                        lhsT=msgt[:, ci, :],
                                        rhs=mt[:, ci * MW : (ci + 1) * MW],
                                        start=(sb == 0 and k == 0),
                                        stop=(sb == NSB - 1 and k == gch - 1),
                                    )
                                    ci += 1
                        assert ci == nch

                    a_sb = [
                        a_pool.tile([128, MW], BF16, tag=f"a{a}",
                                    name=f"a_sb{a}", bufs=2)
                        for a in range(NACC)
                    ]
                    for a in range(NACC):
                        nc.scalar.activation(
                            out=a_sb[a][:], in_=psA[a][:],
                            func=mybir.ActivationFunctionType.Copy,
                        )

                    for t2 in range(B):
                        # transform reuses a late accumulator's bank (its
                        # group is closed and its data copied to SBUF by now)
                        ps = psA[(PAIRS - 1) * B + t2][:, 0:D]
                        for r in range(R):
                            a0 = (r % K_GRP) * 128
                            nc.tensor.matmul(
                                out=ps,
                                lhsT=a_sb[(r // K_GRP) * B + t2][:, a0 : a0 + 128],
                                rhs=w_sb[:, r * D : (r + 1) * D],
                                start=(r == 0),
                                stop=False,
                            )
                        lc = (bat % 2) * 128 if B == 1 else t2 * 128
                        nc.tensor.matmul(
                            out=ps,
                            lhsT=loct[:, lc : lc + 128],
                            rhs=w_sb[:, R * D : (R + 1) * D],
                            start=False,
                            stop=False,
                        )
                        nc.tensor.matmul(
                            out=ps,
                            lhsT=ones_sb[:1, :],
                            rhs=b_sb[:1, :],
                            start=False,
                            stop=True,
                        )
                        row = row0 + t2 * 128
                        if lay == 0:
                            hs = hpool.tile([128, D], BF16, tag="h0")
                            nc.scalar.activation(
                                out=hs[:], in_=ps,
                                func=mybir.ActivationFunctionType.Relu,
                            )
                            nc.sync.dma_start(
                                out=h1b.ap()[row : row + 128, :], in_=hs[:]
                            )
                        else:
                            nrow = min(128, NL - row)
                            if nrow <= 0:
                                continue
                            hs = hpool.tile([128, D], F32, tag="h1")
                            nc.scalar.activation(
                                out=hs[:], in_=ps,
                                func=mybir.ActivationFunctionType.Relu,
                            )
                            nc.sync.dma_start(
                                out=out_d.ap()[row : row + nrow, :],
                                in_=hs[:nrow, :],
                            )

                nc.leave_named_scope(f"lay_{lay}", ls[0], False)
                if lay == 0:
                    nc.gpsimd.collective_compute(
                        "AllGather",
                        mybir.AluOpType.bypass,
                        replica_groups=[list(range(C))],
                        ins=[h1b.ap()],
                        outs=[h1rep.ap()],
                    )

    nc.compile()
    return nc


# ----------------------------------------------------------------------------
# In-map assembly
# ----------------------------------------------------------------------------
def make_in_maps(prep, W1, root1, b1, W2, root2, b2, cfg):
    C, D, R = cfg["C"], cfg["D"], cfg["R"]
    NL, NT, NLP, NTOT, B, NBAT, PAIRS, NSB, SBR, MW = derive(cfg)
    w_all = np.zeros((2, R + 1, D, D), bf16)
    w_all[0, :R] = np.asarray(W1, np.float32).astype(bf16)
    w_all[0, R] = np.asarray(root1, np.float32).astype(bf16)
    w_all[1, :R] = np.asarray(W2, np.float32).astype(bf16)
    w_all[1, R] = np.asarray(root2, np.float32).astype(bf16)
    b_stack = np.stack([np.asarray(b1, np.float32), np.asarray(b2, np.float32)])
    b_all = b_stack.reshape(2, 1, D).astype(bf16)
    ciota = np.tile(np.arange(MW, dtype=np.float32).astype(bf16), (128, 1))

    in_maps = []
    for c in range(C):
        x_loc = np.ascontiguousarray(prep["x_rep"][c * NLP : (c + 1) * NLP])
        im = {
            "x_rep": prep["x_rep"],
            "x_loc": x_loc,
            "w_all": w_all,
            "b_all": b_all,
            "gidx": prep["gidx"][c],
            "dstv": prep["dstv"][c],
            "scal": prep["scal"][c],
            "ciota": ciota,
        }
        if K_MDRAM:
            im["m_w"] = prep["m_w"][c]
        in_maps.append(im)
    return in_maps


def enable_ntff_hook():
    """Register the axon NTFF profiling hook if the image's antenv lacks it."""
    import sys, types
    try:
        import antenv.axon_hooks  # noqa: F401
        return True
    except ImportError:
        pass
    try:
        from trn_agent_boot.trn_boot import _ntff_profile_via_ctypes
        hook = _ntff_profile_via_ctypes("/opt/axon/libaxon_pjrt.so")
        mod = types.ModuleType("antenv.axon_hooks")
        mod._hook = hook
        mod.set_axon_ntff_profile_hook = lambda h: setattr(mod, "_hook", h)
        mod.get_axon_ntff_profile_hook = lambda: mod._hook
        sys.modules["antenv.axon_hooks"] = mod
        import antenv
        antenv.axon_hooks = mod
        return hook is not None
    except Exception:
        return False


_program_cache = {}


def run(x, edge_index, edge_type, W1, root1, b1, W2, root2, b2,
        cfg=FULL, trace=False):
    prep = host_prep(x, edge_index, edge_type, cfg)
    key = (tuple(sorted(cfg.items())), prep["gsz"], prep["runs"], prep["PAD"],
           K_QUEUES, SINGLE_PACKET, MAXC, K_MDRAM, K_GRP, K_B)
    if key not in _program_cache:
        _program_cache[key] = build_program(
            cfg, prep["gsz"], prep["runs"], prep["PAD"]
        )
    nc = _program_cache[key]
    in_maps = make_in_maps(prep, W1, root1, b1, W2, root2, b2, cfg)
    if trace:
        trace = enable_ntff_hook()
    res = run_bass_kernel_spmd(
        nc, in_maps, core_ids=list(range(cfg["C"])), trace=trace
    )
    blocks = [res.results[c]["out"] for c in range(cfg["C"])]
    full = np.concatenate(blocks, axis=0).astype(np.float32)
    return full, res


def kernel(**inputs):
    out, _ = run(
        inputs["x"], inputs["edge_index"], inputs["edge_type"],
        inputs["W1"], inputs["root1"], inputs["b1"],
        inputs["W2"], inputs["root2"], inputs["b2"],
    )
    return out


# revision 29
# speedup vs baseline: 2.8422x; 1.1710x over previous
"""RGCN (2-layer, mean-aggregation) Bass kernel for one TRN2 chip (8 NeuronCores).

Strategy (dst-sharded, matmul-based aggregation — no DRAM scatter):
  - Nodes block-partitioned across 8 cores (12500/core, padded to 12544).
    Edges live on their dst-owner core; x is replicated (bf16, padded layout
    [C*NLP, D]) in every core's HBM.
  - Edges are sorted by (dst tile, src-quarter, rel-group) and padded per
    group to a multiple of 128. Per (tile, src-quarter) run: dma_gather the
    messages x[src] ([128e, D] natural layout), scale by inv_deg (DVE), build
    a one-hot matrix M[e, (rel%GRP)*128 + dst%128] (DVE is_equal against a
    resident iota row, or DMA a host-precomputed fp8 M when K_MDRAM=1), then
    aggregate on the tensor engine:
        A_grpT[d, code] += msg^T @ M   (PSUM, f32 accumulation)
    A_grpT is directly the lhsT for the transform matmuls:
        out[dst, :] = relu(sum_r A_rT.T @ W_r + x_locT.T @ Wroot + b)
  - PSUM: each accumulator is padded to a full 2KB bank (PE start=True zeroes
    the whole 2KB region, so concurrently-open groups must not share a bank);
    accumulators are double-buffered and the transform output aliases into a
    closed accumulator bank.
  - Gathers use int16 indices over 4 equal row-quarters of the replicated
    table (25088 rows < 32768), one SWDGE queue per quarter; deep tile-pool
    buffering (bufs=8) keeps a full tile of 4 gather runs in flight.
  - Between layers one small AllGather (bf16, 3.2MB/rank) replicates the new
    features; layer 1's replication is free (host pre-stages x_rep).
"""

import os
import numpy as np
import ml_dtypes

import concourse.tile as tile
from concourse import bass, bacc, mybir
from concourse.bass_utils import run_bass_kernel_spmd

BF16 = mybir.dt.bfloat16
F32 = mybir.dt.float32
I16 = mybir.dt.int16
bf16 = ml_dtypes.bfloat16

K_QUEUES = int(os.environ.get("K_QUEUES", "4"))
SINGLE_PACKET = os.environ.get("K_SP", "0") == "1"
MAXC = int(os.environ.get("K_MAXC", "1024" if SINGLE_PACKET else "1920"))
# M source: 0 = build one-hot on DVE (is_equal), 1 = DMA precomputed fp8,
# 2 = hybrid (even src-quarters DMA, odd src-quarters DVE)
K_MDRAM = int(os.environ.get("K_MDRAM", "0"))
K_GRP = int(os.environ.get("K_GRP", "2"))   # relations per accumulator group
K_B = int(os.environ.get("K_B", "1"))       # dst tiles per batch
FP8 = mybir.dt.float8e4
fp8 = ml_dtypes.float8_e4m3fn

# ----------------------------------------------------------------------------
# Problem constants
# ----------------------------------------------------------------------------
FULL = dict(N=100000, E=1000000, D=128, R=8, C=8)


def derive(cfg):
    N, D, R, C = cfg["N"], cfg["D"], cfg["R"], cfg["C"]
    NL = N // C                      # owned nodes per core
    NT = (NL + 127) // 128           # dst tiles per core
    NLP = NT * 128                   # padded rows per block
    NTOT = C * NLP                   # replicated-table rows
    B = K_B                          # dst tiles per batch
    NBAT = NT // B                   # batches per core
    PAIRS = R // K_GRP               # accumulator groups per (tile)
    NSB = 4                          # src table quarters
    SBR = NTOT // NSB                # rows per quarter (must be < 32768)
    MW = K_GRP * 128                 # M width: (rel%GRP)*128 + dst%128
    # PSUM: PAIRS*B accumulators, each padded to a full 2KB bank, x2 bufs
    assert PAIRS * B * 2 <= 8 and MW <= 512
    return NL, NT, NLP, NTOT, B, NBAT, PAIRS, NSB, SBR, MW


# ----------------------------------------------------------------------------
# Host-side preprocessing
# ----------------------------------------------------------------------------
def host_prep(x, edge_index, edge_type, cfg):
    N, E, D, R, C = cfg["N"], cfg["E"], cfg["D"], cfg["R"], cfg["C"]
    NL, NT, NLP, NTOT, B, NBAT, PAIRS, NSB, SBR, MW = derive(cfg)
    assert SBR < 32768 and NT % B == 0

    src = np.asarray(edge_index[0], dtype=np.int64)
    dst = np.asarray(edge_index[1], dtype=np.int64)
    et = np.asarray(edge_type, dtype=np.int64)

    # mean-normalization per (relation, dst), computed on host (graph-only)
    deg = np.zeros((R, N), np.float32)
    np.add.at(deg, (et, dst), 1.0)
    inv = np.where(deg > 0, 1.0 / np.maximum(deg, 1.0), 0.0).astype(np.float32)
    scale_e = inv[et, dst]

    core = dst // NL
    dl = dst % NL
    tl = dl // 128
    bat = tl // B
    t2 = tl % B
    pair = et // K_GRP
    code = (et % K_GRP) * 128 + (dl % 128)
    srcp = (src // NL) * NLP + (src % NL)       # padded replicated-table row
    sb = srcp // SBR
    sidx = (srcp % SBR).astype(np.int64)

    NG = NBAT * NSB * PAIRS * B
    g = ((bat * NSB + sb) * PAIRS + pair) * B + t2

    counts = np.zeros((C, NG), np.int64)
    np.add.at(counts, (core, g), 1)
    gsz = np.maximum(((counts.max(axis=0) + 127) // 128) * 128, 128)  # [NG]
    offs = np.zeros(NG + 1, np.int64)
    np.cumsum(gsz, out=offs[1:])
    PAD = int(offs[-1])

    # place edges: stable sort by (core, g), rank within each (core, g) run
    key = core * NG + g
    order = np.argsort(key, kind="stable")
    key_o = key[order]
    new_run = np.ones(E, bool)
    new_run[1:] = key_o[1:] != key_o[:-1]
    run_starts = np.flatnonzero(new_run)
    run_id = np.cumsum(new_run) - 1
    rank = np.arange(E) - run_starts[run_id]
    pos = offs[g[order]] + rank

    gidx_a = np.zeros((C, PAD), np.int16)
    code_i = np.full((C, PAD), 10000, np.int32)   # pad sentinel, out of range
    scal_a = np.zeros((C, PAD), bf16)
    co = core[order]
    gidx_a[co, pos] = sidx[order].astype(np.int16)
    code_i[co, pos] = code[order].astype(np.int32)
    scal_a[co, pos] = scale_e[order].astype(bf16)
    # bf16 codes only feed the is_equal path; bf16 integers are exact to
    # 256 so that path requires MW <= 256
    assert K_MDRAM == 1 or MW <= 256
    code_a = np.minimum(code_i, 300).astype(bf16)

    # wrapped layouts: idx i at [i%16, i//16] (x8 down); code/scale at
    # [i%128, i//128]
    gidx_w = np.tile(
        gidx_a.reshape(C, PAD // 16, 16).transpose(0, 2, 1), (1, 8, 1)
    )
    dstv_w = np.ascontiguousarray(
        code_a.reshape(C, PAD // 128, 128).transpose(0, 2, 1)
    )
    scal_w = np.ascontiguousarray(
        scal_a.reshape(C, PAD // 128, 128).transpose(0, 2, 1)
    )

    # precomputed one-hot M (fp8), wrapped [C, 128, (PAD//128)*MW]
    m_w = None
    if K_MDRAM:
        m_full = (
            code_i.reshape(C, PAD // 128, 128)[..., None]
            == np.arange(MW, dtype=np.int32)
        )
        m_w = np.ascontiguousarray(
            m_full.transpose(0, 2, 1, 3).reshape(C, 128, (PAD // 128) * MW)
        ).astype(fp8)

    # replicated, block-padded x (bf16): [NTOT, D]
    x = np.asarray(x, np.float32)
    x_rep = np.zeros((NTOT, D), bf16)
    for c in range(C):
        x_rep[c * NLP : c * NLP + NL] = x[c * NL : (c + 1) * NL].astype(bf16)

    # run table: per (bat, sb): (offset, size); groups within are (pair, t2)
    runs = []
    for b_ in range(NBAT):
        row = []
        for s_ in range(NSB):
            g0 = ((b_ * NSB + s_) * PAIRS) * B
            o = int(offs[g0])
            n = int(offs[g0 + PAIRS * B] - offs[g0])
            row.append((o, n))
        runs.append(row)

    return dict(
        gsz=tuple(int(v) for v in gsz),
        runs=tuple(tuple(r) for r in runs),
        PAD=PAD,
        gidx=np.ascontiguousarray(gidx_w),
        dstv=dstv_w,
        scal=scal_w,
        m_w=m_w,
        code_i=code_i,
        x_rep=x_rep,
    )


# ----------------------------------------------------------------------------
# Device program
# ----------------------------------------------------------------------------
def build_program(cfg, gsz, runs, PAD):
    N, E, D, R, C = cfg["N"], cfg["E"], cfg["D"], cfg["R"], cfg["C"]
    NL, NT, NLP, NTOT, B, NBAT, PAIRS, NSB, SBR, MW = derive(cfg)

    nc = bacc.Bacc(
        "TRN2", target_bir_lowering=False, debug=False,
        enable_asserts=False, num_devices=C, num_swdge_queues=K_QUEUES,
    )

    x_rep = nc.dram_tensor("x_rep", [NTOT, D], BF16, kind="ExternalInput")
    x_loc = nc.dram_tensor("x_loc", [NLP, D], BF16, kind="ExternalInput")
    w_all = nc.dram_tensor("w_all", [2, R + 1, D, D], BF16, kind="ExternalInput")
    b_all = nc.dram_tensor("b_all", [2, 1, D], BF16, kind="ExternalInput")
    gidx_d = nc.dram_tensor("gidx", [128, PAD // 16], I16, kind="ExternalInput")
    dstv_d = nc.dram_tensor("dstv", [128, PAD // 128], BF16, kind="ExternalInput")
    scal_d = nc.dram_tensor("scal", [128, PAD // 128], BF16, kind="ExternalInput")
    ciota_d = nc.dram_tensor("ciota", [128, MW], BF16, kind="ExternalInput")
    if K_MDRAM:
        m_d = nc.dram_tensor(
            "m_w", [128, (PAD // 128) * MW], FP8, kind="ExternalInput"
        )
    out_d = nc.dram_tensor("out", [NL, D], F32, kind="ExternalOutput")
    h1b = nc.dram_tensor("h1b", [NLP, D], BF16, kind="Internal")
    h1rep = nc.dram_tensor(
        "h1rep", [NTOT, D], BF16, kind="Internal", addr_space="Shared"
    )

    with tile.TileContext(nc) as tc:
        with (
            tc.tile_pool(name="resident", bufs=1) as res_pool,
            tc.tile_pool(name="msg", bufs=8) as msg_pool,
            tc.tile_pool(name="mm", bufs=8) as m_pool,
            tc.tile_pool(name="asb", bufs=2) as a_pool,
            tc.tile_pool(name="loct", bufs=2) as loct_pool,
            tc.tile_pool(name="wpool", bufs=1) as wpool,
            tc.tile_pool(name="hout", bufs=4) as hpool,
            tc.tile_pool(name="psA", bufs=1, space="PSUM") as psA_pool,
        ):
            gidx_sb = res_pool.tile([128, PAD // 16], I16)
            dstv_sb = res_pool.tile([128, PAD // 128], BF16)
            scal_sb = res_pool.tile([128, PAD // 128], BF16)
            ciota_sb = res_pool.tile([128, MW], BF16)
            nc.sync.dma_start(out=gidx_sb[:], in_=gidx_d.ap()[:, :])
            nc.sync.dma_start(out=dstv_sb[:], in_=dstv_d.ap()[:, :])
            nc.sync.dma_start(out=scal_sb[:], in_=scal_d.ap()[:, :])
            nc.sync.dma_start(out=ciota_sb[:], in_=ciota_d.ap()[:, :])
            ones_sb = res_pool.tile([1, D], BF16)
            nc.vector.memset(ones_sb[:], 1.0)

            for lay in range(2):
                src_tab = x_rep if lay == 0 else h1rep
                loc_tab = x_loc if lay == 0 else h1b

                ls = nc.enter_named_scope(f"lay_{lay}", False)
                w_sb = wpool.tile([128, (R + 1) * D], BF16, tag="w", bufs=2)
                nc.sync.dma_start(
                    out=w_sb[:].rearrange("d (r e) -> d r e", r=R + 1),
                    in_=w_all.ap()[lay].rearrange("r d e -> d r e"),
                )
                b_sb = wpool.tile([1, D], BF16, tag="b", bufs=2)
                nc.sync.dma_start(out=b_sb[:], in_=b_all.ap()[lay])

                NACC = PAIRS * B
                for bat in range(NBAT):
                    row0 = bat * B * 128
                    if B > 1 or bat % 2 == 0:
                        nrows = min(max(B, 2) * 128, NLP - row0)
                        loct = loct_pool.tile([128, max(B, 2) * 128], BF16,
                                              tag="lt")
                        nc.sync.dma_start_transpose(
                            out=loct[:, :nrows],
                            in_=loc_tab.ap()[row0 : row0 + nrows, :],
                        )
                    # accumulators, each padded to a full PSUM bank so every
                    # concurrently-open accumulation group owns its own 2KB
                    # zero region (PE start=True zeroes the whole region)
                    psA = [
                        psA_pool.tile([128, MW], F32, tag=f"pa{a}",
                                      name=f"psA{a}", bufs=2,
                                      padded_shape=[128, 512])
                        for a in range(NACC)
                    ]
                    for sb in range(NSB):
                        o, n = runs[bat][sb]
                        nch = n // 128
                        msgt = msg_pool.tile([128, nch, D], BF16, tag="msg")
                        for co in range(0, n, MAXC):
                            cn = min(MAXC, n - co)
                            nc.gpsimd.dma_gather(
                                out_ap=msgt[:, co // 128 : (co + cn) // 128, :],
                                in_ap=src_tab.ap()[sb * SBR : (sb + 1) * SBR, :],
                                idxs_ap=gidx_sb[
                                    :, (o + co) // 16 : (o + co + cn) // 16
                                ],
                                num_idxs=cn,
                                num_idxs_reg=cn,
                                elem_size=D,
                                single_packet=SINGLE_PACKET,
                                queue_num=sb % K_QUEUES,
                            )
                        use_md = K_MDRAM == 1 or (K_MDRAM == 2 and sb % 2 == 0)
                        if use_md:
                            mt = m_pool.tile([128, nch * MW], FP8, tag="mf8")
                            nc.scalar.dma_start(
                                out=mt[:],
                                in_=m_d.ap()[
                                    :, (o // 128) * MW : ((o + n) // 128) * MW
                                ],
                            )
                        else:
                            # is_equal first: it does not depend on the
                            # gather, so it must not sit behind the
                            # gather-gated scale in the DVE queue
                            mt = m_pool.tile([128, nch * MW], BF16, tag="m16")
                            nc.vector.tensor_tensor(
                                out=mt[:].rearrange(
                                    "p (a b) -> p a b", b=MW
                                ),
                                in0=dstv_sb[:, o // 128 : (o + n) // 128, None]
                                .to_broadcast([128, nch, MW]),
                                in1=ciota_sb[:, None, :]
                                .to_broadcast([128, nch, MW]),
                                op=mybir.AluOpType.is_equal,
                            )
                        nc.vector.tensor_tensor(
                            out=msgt[:],
                            in0=msgt[:],
                            in1=scal_sb[:, o // 128 : (o + n) // 128, None]
                            .to_broadcast([128, nch, D]),
                            op=mybir.AluOpType.mult,
                        )
                        ci = 0
                        for p in range(PAIRS):
                            for t2 in range(B):
                                gi = ((bat * NSB + sb) * PAIRS + p) * B + t2
                                gch = gsz[gi] // 128
                                for k in range(gch):
                                    nc.tensor.matmul(
                                        out=psA[p * B + t2][:],
                                        lhsT=msgt[:, ci, :],
                                        rhs=mt[:, ci * MW : (ci + 1) * MW],
                                        start=(sb == 0 and k == 0),
                                        stop=(sb == NSB - 1 and k == gch - 1),
                                    )
                                    ci += 1
                        assert ci == nch

                    a_sb = [
                        a_pool.tile([128, MW], BF16, tag=f"a{a}",
                                    name=f"a_sb{a}", bufs=2)
                        for a in range(NACC)
                    ]
                    for a in range(NACC):
                        nc.scalar.activation(
                            out=a_sb[a][:], in_=psA[a][:],
                            func=mybir.ActivationFunctionType.Copy,
                        )

                    for t2 in range(B):
                        # transform reuses a late accumulator's bank (its
                        # group is closed and its data copied to SBUF by now)
                        ps = psA[(PAIRS - 1) * B + t2][:, 0:D]
                        for r in range(R):
                            a0 = (r % K_GRP) * 128
                            nc.tensor.matmul(
                                out=ps,
                                lhsT=a_sb[(r // K_GRP) * B + t2][:, a0 : a0 + 128],
                                rhs=w_sb[:, r * D : (r + 1) * D],
                                start=(r == 0),
                                stop=False,
                            )
                        lc = (bat % 2) * 128 if B == 1 else t2 * 128
                        nc.tensor.matmul(
                            out=ps,
                            lhsT=loct[:, lc : lc + 128],
                            rhs=w_sb[:, R * D : (R + 1) * D],
                            start=False,
                            stop=False,
                        )
                        nc.tensor.matmul(
                            out=ps,
                            lhsT=ones_sb[:1, :],
                            rhs=b_sb[:1, :],
                            start=False,
                            stop=True,
                        )
                        row = row0 + t2 * 128
                        if lay == 0:
                            hs = hpool.tile([128, D], BF16, tag="h0")
                            nc.scalar.activation(
                                out=hs[:], in_=ps,
                                func=mybir.ActivationFunctionType.Relu,
                            )
                            nc.sync.dma_start(
                                out=h1b.ap()[row : row + 128, :], in_=hs[:]
                            )
                        else:
                            nrow = min(128, NL - row)
                            if nrow <= 0:
                                continue
                            hs = hpool.tile([128, D], F32, tag="h1")
                            nc.scalar.activation(
                                out=hs[:], in_=ps,
                                func=mybir.ActivationFunctionType.Relu,
                            )
                            nc.sync.dma_start(
                                out=out_d.ap()[row : row + nrow, :],
                                in_=hs[:nrow, :],
                            )

                nc.leave_named_scope(f"lay_{lay}", ls[0], False)
                if lay == 0:
                    nc.gpsimd.collective_compute(
                        "AllGather",
                        mybir.AluOpType.bypass,
                        replica_groups=[list(range(C))],
                        ins=[h1b.ap()],
                        outs=[h1rep.ap()],
                    )

    nc.compile()
    return nc


# ----------------------------------------------------------------------------
# In-map assembly
# ----------------------------------------------------------------------------
def make_in_maps(prep, W1, root1, b1, W2, root2, b2, cfg):
    C, D, R = cfg["C"], cfg["D"], cfg["R"]
    NL, NT, NLP, NTOT, B, NBAT, PAIRS, NSB, SBR, MW = derive(cfg)
    w_all = np.zeros((2, R + 1, D, D), bf16)
    w_all[0, :R] = np.asarray(W1, np.float32).astype(bf16)
    w_all[0, R] = np.asarray(root1, np.float32).astype(bf16)
    w_all[1, :R] = np.asarray(W2, np.float32).astype(bf16)
    w_all[1, R] = np.asarray(root2, np.float32).astype(bf16)
    b_stack = np.stack([np.asarray(b1, np.float32), np.asarray(b2, np.float32)])
    b_all = b_stack.reshape(2, 1, D).astype(bf16)
    ciota = np.tile(np.arange(MW, dtype=np.float32).astype(bf16), (128, 1))

    in_maps = []
    for c in range(C):
        x_loc = np.ascontiguousarray(prep["x_rep"][c * NLP : (c + 1) * NLP])
        im = {
            "x_rep": prep["x_rep"],
            "x_loc": x_loc,
            "w_all": w_all,
            "b_all": b_all,
            "gidx": prep["gidx"][c],
            "dstv": prep["dstv"][c],
            "scal": prep["scal"][c],
            "ciota": ciota,
        }
        if K_MDRAM:
            im["m_w"] = prep["m_w"][c]
        in_maps.append(im)
    return in_maps


def enable_ntff_hook():
    """Register the axon NTFF profiling hook if the image's antenv lacks it."""
    import sys, types
    try:
        import antenv.axon_hooks  # noqa: F401
        return True
    except ImportError:
        pass
    try:
        from trn_agent_boot.trn_boot import _ntff_profile_via_ctypes
        hook = _ntff_profile_via_ctypes("/opt/axon/libaxon_pjrt.so")
        mod = types.ModuleType("antenv.axon_hooks")
        mod._hook = hook
        mod.set_axon_ntff_profile_hook = lambda h: setattr(mod, "_hook", h)
        mod.get_axon_ntff_profile_hook = lambda: mod._hook
        sys.modules["antenv.axon_hooks"] = mod
        import antenv
        antenv.axon_hooks = mod
        return hook is not None
    except Exception:
        return False


_program_cache = {}


def run(x, edge_index, edge_type, W1, root1, b1, W2, root2, b2,
        cfg=FULL, trace=False):
    prep = host_prep(x, edge_index, edge_type, cfg)
    key = (tuple(sorted(cfg.items())), prep["gsz"], prep["runs"], prep["PAD"],
           K_QUEUES, SINGLE_PACKET, MAXC, K_MDRAM, K_GRP, K_B)
    if key not in _program_cache:
        _program_cache[key] = build_program(
            cfg, prep["gsz"], prep["runs"], prep["PAD"]
        )
    nc = _program_cache[key]
    in_maps = make_in_maps(prep, W1, root1, b1, W2, root2, b2, cfg)
    if trace:
        trace = enable_ntff_hook()
    res = run_bass_kernel_spmd(
        nc, in_maps, core_ids=list(range(cfg["C"])), trace=trace
    )
    blocks = [res.results[c]["out"] for c in range(cfg["C"])]
    full = np.concatenate(blocks, axis=0).astype(np.float32)
    return full, res


def kernel(**inputs):
    out, _ = run(
        inputs["x"], inputs["edge_index"], inputs["edge_type"],
        inputs["W1"], inputs["root1"], inputs["b1"],
        inputs["W2"], inputs["root2"], inputs["b2"],
    )
    return out
